# revision 15
# baseline (speedup 1.0000x reference)
"""Multitask exact GP posterior on 8 Trainium2 NeuronCores.

Math: with task_K = B B^T + diag(exp(log_var)) (4x4), D = diag(exp(log_noise)),
C = rbf(X,X), the full covariance K = kron(task_K, C) + kron(D, I_N) decouples
through the 4x4 eigendecomposition D^-1/2 task_K D^-1/2 = P diag(lam) P^T into
four independent 1024x1024 SPD solves (I + lam_k C)^-1.  All outputs are then:
  W  = D^-1/2 P,  S = task_K W,  Minv_k = (I + lam_k C)^-1
  T_k = Cx^T Minv_k Cx
  fvar[(i,n),(j,m)] = task_K[i,j]*Cxx[n,m] - sum_k S[i,k]S[j,k] T_k[n,m]
  Z2[:,k] = Minv_k (Y W)[:,k]
  mean_mat = Cx^T (Z2 W^T task_K)   (out4; fmean = vec of its transpose)
  noise = kron(D, I_N)  (diagonal; assembled on host, zero FLOPs)

Device (SPMD, uniform program, per-core data):
  core c in 0..7 handles shift k = c%4 (pairs duplicate) and the 128-column
  test block starting at 128*c.  test_X is rotated by 128*c per core so the
  program stays uniform; the host un-rotates rows when stitching fvar.
  Each core: rbf grams (M = I + lam_k C formed in place, Cx full, Cxx
  128-col slice) -> Schur-recursion inverse (Newton-Schulz 128x128 leaves)
  -> AllGather of the four inverses -> U_k = Minv_k [Cx[:, :128] | Z1]
  -> T_k = Cx^T U_k -> fvar stripes + mean slice.
"""
import sys

if "/opt/trn_rl_repo" not in sys.path:
    sys.path.insert(0, "/opt/trn_rl_repo")

import numpy as np

import concourse.bass as bass
import concourse.bass_isa as bass_isa
import concourse.mybir as mybir
import concourse.tile as tile
from concourse import bacc, bass_utils
from concourse.masks import make_identity

f32 = np.float32
FP = mybir.dt.float32
N = 1024          # train points
NT = 1024         # test points
D_IN = 8          # input dim
L = 4             # tasks
NC = 8            # cores
PT = 8            # 128-row tiles per 1024
P = 128
NS_ITERS = 10
UW = 132          # U columns: 128 test cols + 4 Z1 cols
UTW = UW + P      # combined U|T row width

# consts tensor column layout (per-core [128, CN] fp32, value broadcast
# down the partition axis so any column slices as a [128,1] scalar AP)
C_LAM = 0         # lam_k for this core
C_EXPSC = 1       # -0.5 / lengthscale^2
C_TK = 2          # 16: task_K[i,j] at 2 + 4*i + j
C_SC = 18         # 64: -S[i,k]*S[j,k] at 18 + 16*i + 4*j + k
C_WT2 = 82        # 16: (W^T task_K)[k,i] at 82 + 4*k + i
CN = 98

MULT = mybir.AluOpType.mult
ADD = mybir.AluOpType.add


class MV:
    """Block view of a [128, ntiles, ncols] SBUF tile holding a matrix whose
    element (rt*128+p, c) lives at ap[p, r0+rt, c0+c]."""

    def __init__(self, ap, r0=0, c0=0):
        self.ap = ap
        self.r0 = r0
        self.c0 = c0

    def sub(self, r, c):
        return MV(self.ap, self.r0 + r // P, self.c0 + c)

    def blk(self, i, j):
        return self.ap[:, self.r0 + i, self.c0 + j * P:self.c0 + (j + 1) * P]

    def row(self, i, c_start, w):
        return self.ap[:, self.r0 + i, self.c0 + c_start:self.c0 + c_start + w]


def _build_program(dump_minv=False):
    nc = bacc.Bacc("TRN2", target_bir_lowering=False, debug=False,
                   num_devices=NC)

    # I/O
    i_xt = nc.dram_tensor("xt", [D_IN, N], FP, kind="ExternalInput").ap()
    i_txt = nc.dram_tensor("txt", [D_IN, NT], FP, kind="ExternalInput").ap()
    i_x2c = nc.dram_tensor("x2c", [P, PT], FP, kind="ExternalInput").ap()
    i_x2r = nc.dram_tensor("x2r", [1, N], FP, kind="ExternalInput").ap()
    i_tx2c = nc.dram_tensor("tx2c", [P, PT], FP, kind="ExternalInput").ap()
    i_tx2r = nc.dram_tensor("tx2r", [1, NT], FP, kind="ExternalInput").ap()
    i_z1 = nc.dram_tensor("z1", [N, L], FP, kind="ExternalInput").ap()
    i_cst = nc.dram_tensor("cst", [P, CN], FP, kind="ExternalInput").ap()
    o_fvar = nc.dram_tensor("fvar_part", [L * NT, L * P], FP,
                            kind="ExternalOutput").ap()
    o_mean = nc.dram_tensor("mean_part", [P, L], FP, kind="ExternalOutput").ap()

    ag_in = nc.dram_tensor("ag_in", [N, N], FP)
    ag_out = nc.dram_tensor("ag_out", [L * N, N], FP)
    o_dbg = None
    o_dbg2 = None
    dbg_rows = [0]
    dbg_names = []
    if dump_minv:
        o_dbg = nc.dram_tensor("dbg_mm", [2 * N, N], FP,
                               kind="ExternalOutput").ap()
        o_dbg2 = nc.dram_tensor("dbg_probe", [40 * P, 512], FP,
                                kind="ExternalOutput").ap()

    with tile.TileContext(nc) as tc:
        with (
            tc.tile_pool(name="const", bufs=1) as consts,
            tc.tile_pool(name="big", bufs=1) as big,
            tc.tile_pool(name="work", bufs=1) as work,
            tc.tile_pool(name="ns", bufs=2) as nsp,
            tc.tile_pool(name="ps512", bufs=2, space="PSUM") as ps512,
            tc.tile_pool(name="ps132", bufs=2, space="PSUM") as ps132,
            tc.tile_pool(name="ps128", bufs=4, space="PSUM") as ps128,
            tc.tile_pool(name="stream", bufs=6) as stream,
            tc.tile_pool(name="acc", bufs=6) as accp,
        ):
            def mm512(lhsT, rhs, evict, m_tiles, n_cols, k_tiles):
                """out[mt, c0:c0+w] = sum_kt lhsT.blk(kt,mt)^T @ rhs.row(...)"""
                for mt in range(m_tiles):
                    for c0 in range(0, n_cols, 512):
                        w = min(512, n_cols - c0)
                        ps = ps512.tile([P, 512], FP, tag="s512")
                        for kt in range(k_tiles):
                            nc.tensor.matmul(ps[:, :w], lhsT.blk(kt, mt),
                                             rhs.row(kt, c0, w),
                                             start=(kt == 0),
                                             stop=(kt == k_tiles - 1))
                        evict(ps[:, :w], mt, c0, w)

            # ---- constants ----
            ident = consts.tile([P, P], FP)
            make_identity(nc, ident)
            ident2 = consts.tile([P, P], FP)
            nc.vector.tensor_scalar_mul(ident2[:], ident[:], 2.0)
            cst = consts.tile([P, CN], FP)
            nc.sync.dma_start(cst[:], i_cst)

            def cs(col):
                return cst[:, col:col + 1]

            xt = consts.tile([D_IN, N], FP)
            nc.sync.dma_start(xt[:], i_xt)
            txt = consts.tile([D_IN, NT], FP)
            nc.sync.dma_start(txt[:], i_txt)
            x2c = consts.tile([P, PT], FP)
            nc.sync.dma_start(x2c[:], i_x2c)
            tx2c = consts.tile([P, PT], FP)
            nc.sync.dma_start(tx2c[:], i_tx2c)
            x2r1 = consts.tile([1, N], FP)
            nc.sync.dma_start(x2r1[:], i_x2r)
            tx2r1 = consts.tile([1, NT], FP)
            nc.sync.dma_start(tx2r1[:], i_tx2r)
            x2rb = consts.tile([P, N], FP)
            nc.gpsimd.partition_broadcast(x2rb[:], x2r1[:])
            tx2rb = consts.tile([P, NT], FP)
            nc.gpsimd.partition_broadcast(tx2rb[:], tx2r1[:])
            z1 = consts.tile([P, PT, L], FP)
            for t in range(PT):
                nc.sync.dma_start(z1[:, t, :], i_z1[t * P:(t + 1) * P, :])

            # ---- rbf grams;  M = lam * rbf(X,X) + I formed in place ----
            mmat = big.tile([P, PT, N], FP)     # M, then its inverse's source
            cx = big.tile([P, PT, NT], FP)      # Cx = rbf(X, test_rot)
            cxx = work.tile([P, PT, P], FP)     # Cxx[:, first 128 rot cols]

            def emit_gram(dst, t, lhs_cols, rhs_all, col_tile, row_b, n_cols):
                for c0 in range(0, n_cols, 512):
                    w = min(512, n_cols - c0)
                    ps = ps512.tile([P, 512], FP, tag="s512")
                    nc.tensor.matmul(ps[:, :w],
                                     lhs_cols[:, t * P:(t + 1) * P],
                                     rhs_all[:, c0:c0 + w],
                                     start=True, stop=True)
                    d2 = dst[:, t, c0:c0 + w]
                    nc.vector.scalar_tensor_tensor(
                        d2, ps[:, :w], -2.0, row_b[:, c0:c0 + w],
                        op0=MULT, op1=ADD)
                    nc.vector.tensor_scalar_add(d2, d2, col_tile[:, t:t + 1])
                    nc.scalar.activation(d2, d2,
                                         mybir.ActivationFunctionType.Exp,
                                         scale=cs(C_EXPSC))

            for t in range(PT):
                emit_gram(mmat, t, xt, xt, x2c, x2rb, N)
                nc.vector.tensor_scalar_mul(mmat[:, t, :], mmat[:, t, :],
                                            cs(C_LAM))
                nc.vector.tensor_add(mmat[:, t, t * P:(t + 1) * P],
                                     mmat[:, t, t * P:(t + 1) * P], ident[:])
            for t in range(PT):
                emit_gram(cx, t, xt, txt, x2c, tx2rb, NT)
            for t in range(PT):
                emit_gram(cxx, t, txt, txt[:, 0:P], tx2c, tx2rb[:, 0:P], P)

            # ---- Schur-recursion inverse with Newton-Schulz leaves ----
            minv = big.tile([P, PT, N], FP, tag="islot")

            def dbg(name, ap, w):
                if not dump_minv or dbg_rows[0] >= 40:
                    return
                r = dbg_rows[0]
                dbg_rows[0] += 1
                dbg_names.append((name, w))
                nc.sync.dma_start(o_dbg2[r * P:(r + 1) * P, 0:w], ap)
            nc._dbg_names = dbg_names

            def emit_ns(a_blk, o_blk):
                rs = nsp.tile([P, 1], FP, tag="ns_rs")
                nc.vector.tensor_reduce(rs[:], a_blk, mybir.AxisListType.X,
                                        ADD, apply_absolute_value=True)
                mx = nsp.tile([P, 1], FP, tag="ns_mx")
                nc.gpsimd.partition_all_reduce(mx[:], rs[:], 128,
                                               bass_isa.ReduceOp.max)
                cc = nsp.tile([P, 1], FP, tag="ns_cc")
                nc.vector.reciprocal(cc[:], mx[:])
                # X0 = cc * (2I - cc*A)
                xcur = nsp.tile([P, P], FP, tag="ns_x")
                t0 = nsp.tile([P, P], FP, tag="ns_t0")
                nc.vector.tensor_scalar_mul(t0[:], a_blk, cc[:])
                nc.vector.scalar_tensor_tensor(t0[:], t0[:], -1.0, ident2[:],
                                               op0=MULT, op1=ADD)
                nc.vector.tensor_scalar_mul(xcur[:], t0[:], cc[:])
                for it in range(NS_ITERS - 1):
                    psp = ps128.tile([P, P], FP, tag="mm128")
                    nc.tensor.matmul(psp[:], a_blk, xcur[:],
                                     start=True, stop=True)
                    g = nsp.tile([P, P], FP, tag="ns_g")
                    nc.vector.scalar_tensor_tensor(g[:], psp[:], -1.0,
                                                   ident2[:], op0=MULT,
                                                   op1=ADD)
                    psx = ps128.tile([P, P], FP, tag="mm128")
                    nc.tensor.matmul(psx[:], xcur[:], g[:],
                                     start=True, stop=True)
                    if it == NS_ITERS - 2:
                        nc.vector.tensor_copy(o_blk, psx[:])
                    else:
                        xnxt = nsp.tile([P, P], FP, tag="ns_x")
                        nc.vector.tensor_copy(xnxt[:], psx[:])
                        xcur = xnxt

            def emit_transpose_inplace(G, ht):
                # G <- G^T, block-pair swaps staged through PSUM
                for i in range(ht):
                    for j in range(i, ht):
                        ps1 = ps128.tile([P, P], FP, tag="mm128")
                        nc.tensor.transpose(ps1[:], G.blk(i, j), ident[:])
                        if i == j:
                            nc.scalar.copy(G.blk(i, i), ps1[:])
                        else:
                            ps2 = ps128.tile([P, P], FP, tag="mm128")
                            nc.tensor.transpose(ps2[:], G.blk(j, i), ident[:])
                            nc.scalar.copy(G.blk(j, i), ps1[:])
                            nc.scalar.copy(G.blk(i, j), ps2[:])

            def emit_inv(n, mv, ov, path="R"):
                if n == P:
                    if path in ("R.A.A.A", "R.S.A.A"):
                        dbg(f"{path}.leafA", mv.blk(0, 0), P)
                    emit_ns(mv.blk(0, 0), ov.blk(0, 0))
                    if path in ("R.A.A.A", "R.S.A.A"):
                        dbg(f"{path}.leafO", ov.blk(0, 0), P)
                    return
                h = n // 2
                ht = h // P
                A, B, Dd = mv.sub(0, 0), mv.sub(0, h), mv.sub(h, h)
                ai_t = work.tile([P, ht, h], FP, tag=f"ai{n}")
                g_t = work.tile([P, ht, h], FP, tag=f"g{n}")
                sh_t = work.tile([P, ht, h], FP, tag=f"sh{n}")
                si_t = work.tile([P, ht, h], FP, tag=f"si{n}")
                Ai, G = MV(ai_t), MV(g_t)
                S, Si = MV(sh_t), MV(si_t)

                emit_inv(h, A, Ai, path + ".A")
                # G = Ai @ B
                mm512(Ai, B,
                      lambda ps, mt, c0, w: nc.scalar.copy(G.row(mt, c0, w),
                                                           ps),
                      ht, h, ht)
                # iterative refinement: G += Ai (B - A G).  Keeps the Schur
                # complement accurate even though each NS inverse only
                # reaches its own kappa*eps floor; without it the ||B||^2
                # amplification makes the top-level S indefinite in fp32.
                Rv = MV(sh_t)  # S's storage is free until S is formed
                for _ in range(2 if n >= 512 else 1):
                    mm512(A, G,
                          lambda ps, mt, c0, w: nc.vector.scalar_tensor_tensor(
                              Rv.row(mt, c0, w), ps, -1.0, B.row(mt, c0, w),
                              op0=MULT, op1=ADD),
                          ht, h, ht)
                    mm512(Ai, Rv,
                          lambda ps, mt, c0, w: nc.vector.tensor_add(
                              G.row(mt, c0, w), ps, G.row(mt, c0, w)),
                          ht, h, ht)
                if path in ("R.A", "R.A.A", "R", "R.S"):
                    dbg(f"{path}.Ai", Ai.blk(0, 0), P)
                    dbg(f"{path}.G", G.blk(0, 0), P)
                # S = D - B^T G
                mm512(B, G,
                      lambda ps, mt, c0, w: nc.vector.scalar_tensor_tensor(
                          S.row(mt, c0, w), ps, -1.0, Dd.row(mt, c0, w),
                          op0=MULT, op1=ADD),
                      ht, h, ht)
                if path in ("R.A", "R.A.A", "R", "R.S"):
                    dbg(f"{path}.S", S.blk(0, 0), P)
                emit_inv(h, S, Si, path + ".S")
                if path in ("R.A", "R.A.A", "R", "R.S"):
                    dbg(f"{path}.Si", Si.blk(0, 0), P)
                # G <- G^T  (S dead -> its tile becomes H's storage)
                emit_transpose_inplace(G, ht)
                Gt = G
                H = MV(sh_t)
                if path in ("R.A", "R.A.A", "R", "R.S"):
                    dbg(f"{path}.Gt", Gt.blk(0, 0), P)
                # H = Si @ G^T
                mm512(Si, Gt,
                      lambda ps, mt, c0, w: nc.scalar.copy(H.row(mt, c0, w),
                                                           ps),
                      ht, h, ht)
                if path in ("R.A", "R.A.A", "R", "R.S"):
                    dbg(f"{path}.H", H.blk(0, 0), P)
                # TL = Ai + G @ H   (lhsT for G@H is Gt)
                mm512(Gt, H,
                      lambda ps, mt, c0, w: nc.vector.tensor_add(
                          ov.row(mt, c0, w), ps, Ai.row(mt, c0, w)),
                      ht, h, ht)
                if path in ("R.A", "R.A.A", "R", "R.S"):
                    dbg(f"{path}.TL", ov.blk(0, 0), P)
                # TR = -(G @ Si)
                mm512(Gt, Si,
                      lambda ps, mt, c0, w: nc.vector.tensor_scalar_mul(
                          ov.sub(0, h).row(mt, c0, w), ps, -1.0),
                      ht, h, ht)
                # BL = -H ; BR = Si
                for mt in range(ht):
                    nc.scalar.mul(ov.sub(h, 0).row(mt, 0, h),
                                  H.row(mt, 0, h), -1.0)
                    nc.scalar.copy(ov.sub(h, h).row(mt, 0, h),
                                   Si.row(mt, 0, h))

            if dump_minv:
                for t in range(PT):
                    nc.sync.dma_start(o_dbg[t * P:(t + 1) * P, :],
                                      mmat[:, t, :])

            emit_inv(N, MV(mmat), MV(minv))

            if dump_minv:
                for t in range(PT):
                    nc.sync.dma_start(o_dbg[N + t * P:N + (t + 1) * P, :],
                                      minv[:, t, :])

            # ---- AllGather the four inverses ----
            for t in range(PT):
                nc.sync.dma_start(ag_in.ap()[t * P:(t + 1) * P, :],
                                  minv[:, t, :])
            nc.gpsimd.collective_compute(
                "AllGather", mybir.AluOpType.bypass,
                replica_groups=[[0, 1, 2, 3], [4, 5, 6, 7]],
                ins=[ag_in.ap().opt()],
                outs=[ag_out.ap().opt()],
            )

            # ---- CxZ = [Cx[:, 0:128] | Z1] ----
            cxz = work.tile([P, PT, UW], FP)
            for t in range(PT):
                nc.vector.tensor_copy(cxz[:, t, 0:P], cx[:, t, 0:P])
                nc.vector.tensor_copy(cxz[:, t, P:UW], z1[:, t, :])

            # ---- U_k = Minv_k @ CxZ ;  T_k = Cx^T @ U_k[:, :128] ----
            # combined tile [*, k*PT+mt, 0:UW]=U, [*, k*PT+mt, UW:UTW]=T;
            # shares the SBUF slot of minv (dead after the AllGather).
            ut = big.tile([P, L * PT, UTW], FP, tag="islot")
            for k in range(L):
                for mt in range(PT):
                    ps = ps132.tile([P, UW], FP, tag="umm")
                    for kt in range(PT):
                        mv_blk = stream.tile([P, P], FP, tag="minv_dma")
                        nc.sync.dma_start(
                            mv_blk[:],
                            ag_out.ap()[k * N + kt * P:k * N + (kt + 1) * P,
                                        mt * P:(mt + 1) * P])
                        nc.tensor.matmul(ps[:], mv_blk[:], cxz[:, kt, :],
                                         start=(kt == 0), stop=(kt == PT - 1))
                    nc.scalar.copy(ut[:, k * PT + mt, 0:UW], ps[:])
                for mt in range(PT):
                    ps = ps128.tile([P, P], FP, tag="mm128")
                    for kt in range(PT):
                        nc.tensor.matmul(ps[:],
                                         cx[:, kt, mt * P:(mt + 1) * P],
                                         ut[:, k * PT + kt, 0:P],
                                         start=(kt == 0), stop=(kt == PT - 1))
                    nc.scalar.copy(ut[:, k * PT + mt, UW:UTW], ps[:])

            # ---- V = Z2 @ (W^T task_K) ;  mean = Cx[:, :128]^T @ V ----
            v = work.tile([P, PT, L], FP)
            for i in range(L):
                for k in range(L):
                    z2k = ut[:, k * PT:(k + 1) * PT, P + k]
                    if k == 0:
                        nc.vector.tensor_scalar_mul(v[:, :, i], z2k,
                                                    cs(C_WT2 + 4 * k + i))
                    else:
                        nc.vector.scalar_tensor_tensor(
                            v[:, :, i], z2k, cs(C_WT2 + 4 * k + i), v[:, :, i],
                            op0=MULT, op1=ADD)
            psm = ps128.tile([P, L], FP, tag="mm128")
            for kt in range(PT):
                nc.tensor.matmul(psm[:], cx[:, kt, 0:P], v[:, kt, :],
                                 start=(kt == 0), stop=(kt == PT - 1))
            meansb = work.tile([P, L], FP)
            nc.vector.tensor_copy(meansb[:], psm[:])
            nc.sync.dma_start(o_mean, meansb[:])

            # ---- fvar stripes ----
            for i in range(L):
                for j in range(L):
                    for t in range(PT):
                        acc = accp.tile([P, P], FP, tag="acc")
                        nc.scalar.mul(acc[:], cxx[:, t, :],
                                      cs(C_TK + 4 * i + j))
                        for k in range(L):
                            eng = nc.vector
                            eng.scalar_tensor_tensor(
                                acc[:], ut[:, k * PT + t, UW:UTW],
                                cs(C_SC + 16 * i + 4 * j + k), acc[:],
                                op0=MULT, op1=ADD)
                        nc.sync.dma_start(
                            o_fvar[i * NT + t * P:i * NT + (t + 1) * P,
                                   j * P:(j + 1) * P], acc[:])

    nc.compile()
    return nc


_NC_CACHE = [None]


def _get_program():
    if _NC_CACHE[0] is None:
        _NC_CACHE[0] = _build_program()
    return _NC_CACHE[0]


def _host_prep(X, test_X, Y, log_noise, covar_factor, log_var, log_lengthscale):
    X = np.asarray(X, f32)
    test_X = np.asarray(test_X, f32)
    Y = np.asarray(Y, f32)
    log_noise = np.asarray(log_noise, f32)
    covar_factor = np.asarray(covar_factor, f32)
    log_var = np.asarray(log_var, f32)
    ls = float(np.asarray(log_lengthscale, f32))

    task_K = (covar_factor @ covar_factor.T
              + np.diag(np.exp(log_var))).astype(f32)
    d = np.exp(log_noise).astype(f32)
    dih = (1.0 / np.sqrt(d)).astype(f32)
    lam, Pm = np.linalg.eigh(
        (dih[:, None] * task_K * dih[None, :]).astype(np.float64))
    lam = lam.astype(f32)
    Pm = Pm.astype(f32)
    W = (dih[:, None] * Pm).astype(f32)
    Smat = (task_K @ W).astype(f32)
    Wt2 = (W.T @ task_K).astype(f32)
    Z1 = (Y @ W).astype(f32)
    expsc = f32(-0.5 * np.exp(-2.0 * ls))

    x2 = np.sum(X * X, axis=1).astype(f32)
    tx2 = np.sum(test_X * test_X, axis=1).astype(f32)

    in_maps = []
    for c in range(NC):
        k = c % 4
        perm = np.roll(np.arange(NT), -P * c)
        trot = test_X[perm]
        tx2rot = tx2[perm]
        cstv = np.zeros((CN,), f32)
        cstv[C_LAM] = lam[k]
        cstv[C_EXPSC] = expsc
        for i in range(L):
            for j in range(L):
                cstv[C_TK + 4 * i + j] = task_K[i, j]
                for kk in range(L):
                    cstv[C_SC + 16 * i + 4 * j + kk] = \
                        -Smat[i, kk] * Smat[j, kk]
        for kk in range(L):
            for i in range(L):
                cstv[C_WT2 + 4 * kk + i] = Wt2[kk, i]
        in_maps.append({
            "xt": np.ascontiguousarray(X.T),
            "txt": np.ascontiguousarray(trot.T),
            "x2c": np.ascontiguousarray(x2.reshape(PT, P).T),
            "x2r": x2.reshape(1, N).copy(),
            "tx2c": np.ascontiguousarray(tx2rot.reshape(PT, P).T),
            "tx2r": tx2rot.reshape(1, NT).copy(),
            "z1": Z1,
            "cst": np.broadcast_to(cstv, (P, CN)).copy(),
        })
    return in_maps, d


def _stitch(results, d):
    fvar = np.empty((L * NT, L * NT), f32)
    mean = np.empty((NT, L), f32)
    for c in range(NC):
        fp = results[c]["fvar_part"]
        mean[c * P:(c + 1) * P, :] = results[c]["mean_part"]
        for i in range(L):
            blk = np.roll(fp[i * NT:(i + 1) * NT, :], P * c, axis=0)
            for j in range(L):
                fvar[i * NT:(i + 1) * NT,
                     j * NT + c * P:j * NT + (c + 1) * P] = \
                    blk[:, j * P:(j + 1) * P]
    noise = np.zeros((L * N, L * N), f32)
    noise[np.arange(L * N), np.arange(L * N)] = np.repeat(d, N)
    fmean = mean.T.reshape(-1, 1).copy()
    return fmean, fvar, noise, mean


def kernel(**inputs):
    nc = _get_program()
    in_maps, d = _host_prep(**inputs)
    res = bass_utils.run_bass_kernel_spmd(nc, in_maps,
                                          core_ids=list(range(NC)))
    return _stitch(res.results, d)


# revision 20
# speedup vs baseline: 1.0420x; 1.0420x over previous
"""Multitask exact GP posterior on 8 Trainium2 NeuronCores.

Math: with task_K = B B^T + diag(exp(log_var)) (4x4), D = diag(exp(log_noise)),
C = rbf(X,X), the full covariance K = kron(task_K, C) + kron(D, I_N) decouples
through the 4x4 eigendecomposition D^-1/2 task_K D^-1/2 = P diag(lam) P^T into
four independent 1024x1024 SPD solves (I + lam_k C)^-1.  All outputs are then:
  W  = D^-1/2 P,  S = task_K W,  Minv_k = (I + lam_k C)^-1
  T_k = Cx^T Minv_k Cx
  fvar[(i,n),(j,m)] = task_K[i,j]*Cxx[n,m] - sum_k S[i,k]S[j,k] T_k[n,m]
  Z2[:,k] = Minv_k (Y W)[:,k]
  mean_mat = Cx^T (Z2 W^T task_K)   (out4; fmean = vec of its transpose)
  noise = kron(D, I_N)  (diagonal; assembled on host, zero FLOPs)

Device (SPMD, uniform program, per-core data):
  core c in 0..7 handles shift k = c%4 (pairs duplicate) and the 128-column
  test block starting at 128*c.  test_X is rotated by 128*c per core so the
  program stays uniform; the host un-rotates rows when stitching fvar.
  Each core: rbf grams (M = I + lam_k C formed in place, Cx full, Cxx
  128-col slice) -> Schur-recursion inverse (Newton-Schulz 128x128 leaves,
  per-level iterative refinement of G = A^-1 B) -> 8-rank AllGather of the
  inverses -> U_k = Minv_k [Cx[:, :128] | Z1] -> T_k = Cx^T U_k ->
  fvar upper-triangle task stripes (host mirrors the rest) + mean slice.
"""
import sys

if "/opt/trn_rl_repo" not in sys.path:
    sys.path.insert(0, "/opt/trn_rl_repo")

import numpy as np

import concourse.bass_isa as bass_isa
import concourse.mybir as mybir
import concourse.tile as tile
from concourse import bacc, bass_utils
from concourse.masks import make_identity

f32 = np.float32
FP = mybir.dt.float32
N = 1024          # train points
NT = 1024         # test points
D_IN = 8          # input dim
L = 4             # tasks
NC = 8            # cores
PT = 8            # 128-row tiles per 1024
P = 128
NS_ITERS = 10
UW = 132          # U columns: 128 test cols + 4 Z1 cols
UTW = UW + P      # combined U|T row width

# consts tensor column layout (per-core [128, CN] fp32, value broadcast
# down the partition axis so any column slices as a [128,1] scalar AP)
C_LAM = 0         # lam_k for this core
C_EXPSC = 1       # -0.5 / lengthscale^2
C_TK = 2          # 16: task_K[i,j] at 2 + 4*i + j
C_SC = 18         # 64: -S[i,k]*S[j,k] at 18 + 16*i + 4*j + k
C_WT2 = 82        # 16: (W^T task_K)[k,i] at 82 + 4*k + i
CN = 98

MULT = mybir.AluOpType.mult
ADD = mybir.AluOpType.add


class MV:
    """Block view of a [128, ntiles, ncols] SBUF tile holding a matrix whose
    element (rt*128+p, c) lives at ap[p, r0+rt, c0+c]."""

    def __init__(self, ap, r0=0, c0=0):
        self.ap = ap
        self.r0 = r0
        self.c0 = c0

    def sub(self, r, c):
        return MV(self.ap, self.r0 + r // P, self.c0 + c)

    def blk(self, i, j):
        return self.ap[:, self.r0 + i, self.c0 + j * P:self.c0 + (j + 1) * P]

    def row(self, i, c_start, w):
        return self.ap[:, self.r0 + i, self.c0 + c_start:self.c0 + c_start + w]


def _build_program(dump_minv=False):
    nc = bacc.Bacc("TRN2", target_bir_lowering=False, debug=False,
                   num_devices=NC)

    # I/O
    i_xt = nc.dram_tensor("xt", [D_IN, N], FP, kind="ExternalInput").ap()
    i_txt = nc.dram_tensor("txt", [D_IN, NT], FP, kind="ExternalInput").ap()
    i_x2c = nc.dram_tensor("x2c", [P, PT], FP, kind="ExternalInput").ap()
    i_x2r = nc.dram_tensor("x2r", [1, N], FP, kind="ExternalInput").ap()
    i_tx2c = nc.dram_tensor("tx2c", [P, PT], FP, kind="ExternalInput").ap()
    i_tx2r = nc.dram_tensor("tx2r", [1, NT], FP, kind="ExternalInput").ap()
    i_z1 = nc.dram_tensor("z1", [N, L], FP, kind="ExternalInput").ap()
    i_cst = nc.dram_tensor("cst", [P, CN], FP, kind="ExternalInput").ap()
    o_fvar = nc.dram_tensor("fvar_part", [L * NT, L * P], FP,
                            kind="ExternalOutput").ap()
    o_mean = nc.dram_tensor("mean_part", [P, L], FP, kind="ExternalOutput").ap()

    ag_in = nc.dram_tensor("ag_in", [N, N], FP)
    ag_out = nc.dram_tensor("ag_out", [NC * N, N], FP)
    o_dbg = None
    if dump_minv:
        o_dbg = nc.dram_tensor("dbg_mm", [2 * N, N], FP,
                               kind="ExternalOutput").ap()

    with tile.TileContext(nc) as tc:
        with (
            tc.tile_pool(name="const", bufs=1) as consts,
            tc.tile_pool(name="big", bufs=1) as big,
            tc.tile_pool(name="work", bufs=1) as work,
            tc.tile_pool(name="ns", bufs=2) as nsp,
            tc.tile_pool(name="mstr", bufs=2) as mstr,
            tc.tile_pool(name="ps512", bufs=2, space="PSUM") as ps512,
            tc.tile_pool(name="ps132", bufs=2, space="PSUM") as ps132,
            tc.tile_pool(name="ps128", bufs=4, space="PSUM") as ps128,
            tc.tile_pool(name="acc", bufs=2) as accp,
        ):
            def mm512(lhsT, rhs, evict, m_tiles, n_cols, k_tiles):
                """out[mt, c0:c0+w] = sum_kt lhsT.blk(kt,mt)^T @ rhs.row(...)"""
                for mt in range(m_tiles):
                    for c0 in range(0, n_cols, 512):
                        w = min(512, n_cols - c0)
                        ps = ps512.tile([P, 512], FP, tag="s512")
                        for kt in range(k_tiles):
                            nc.tensor.matmul(ps[:, :w], lhsT.blk(kt, mt),
                                             rhs.row(kt, c0, w),
                                             start=(kt == 0),
                                             stop=(kt == k_tiles - 1))
                        evict(ps[:, :w], mt, c0, w)

            # ---- constants ----
            ident = consts.tile([P, P], FP)
            make_identity(nc, ident)
            ident2 = consts.tile([P, P], FP)
            nc.vector.tensor_scalar_mul(ident2[:], ident[:], 2.0)
            cst = consts.tile([P, CN], FP)
            nc.sync.dma_start(cst[:], i_cst)

            def cs(col):
                return cst[:, col:col + 1]

            xt = consts.tile([D_IN, N], FP)
            nc.sync.dma_start(xt[:], i_xt)
            txt = consts.tile([D_IN, NT], FP)
            nc.sync.dma_start(txt[:], i_txt)
            x2c = consts.tile([P, PT], FP)   # pre-scaled by expsc (host)
            nc.sync.dma_start(x2c[:], i_x2c)
            tx2c = consts.tile([P, PT], FP)  # pre-scaled by expsc (host)
            nc.sync.dma_start(tx2c[:], i_tx2c)
            x2r1 = consts.tile([1, N], FP)
            nc.sync.dma_start(x2r1[:], i_x2r)
            tx2r1 = consts.tile([1, NT], FP)
            nc.sync.dma_start(tx2r1[:], i_tx2r)
            x2rb = consts.tile([P, N], FP)
            nc.gpsimd.partition_broadcast(x2rb[:], x2r1[:])
            tx2rb = consts.tile([P, NT], FP)
            nc.gpsimd.partition_broadcast(tx2rb[:], tx2r1[:])
            z1 = consts.tile([P, PT, L], FP)
            for t in range(PT):
                nc.sync.dma_start(z1[:, t, :], i_z1[t * P:(t + 1) * P, :])

            # ---- rbf grams;  M = lam * rbf(X,X) + I formed in place ----
            mmat = big.tile([P, PT, N], FP)     # M for this core's shift
            cx = big.tile([P, PT, NT], FP)      # Cx = rbf(X, test_rot)
            cxx = work.tile([P, PT, P], FP)     # Cxx[:, first 128 rot cols]

            def emit_gram(dst, t, lhs_cols, rhs_all, col_bias, row_b, n_cols):
                # dst[t] = exp(expsc*(-2*lhs^T rhs + rowv) + col_bias)
                for c0 in range(0, n_cols, 512):
                    w = min(512, n_cols - c0)
                    ps = ps512.tile([P, 512], FP, tag="s512")
                    nc.tensor.matmul(ps[:, :w],
                                     lhs_cols[:, t * P:(t + 1) * P],
                                     rhs_all[:, c0:c0 + w],
                                     start=True, stop=True)
                    d2 = dst[:, t, c0:c0 + w]
                    nc.vector.scalar_tensor_tensor(
                        d2, ps[:, :w], -2.0, row_b[:, c0:c0 + w],
                        op0=MULT, op1=ADD)
                    nc.scalar.activation(d2, d2,
                                         mybir.ActivationFunctionType.Exp,
                                         scale=cs(C_EXPSC), bias=col_bias)

            for t in range(PT):
                emit_gram(mmat, t, xt, xt, x2c[:, t:t + 1], x2rb, N)
                nc.vector.tensor_scalar_mul(mmat[:, t, :], mmat[:, t, :],
                                            cs(C_LAM))
                nc.vector.tensor_add(mmat[:, t, t * P:(t + 1) * P],
                                     mmat[:, t, t * P:(t + 1) * P], ident[:])
            for t in range(PT):
                emit_gram(cx, t, xt, txt, x2c[:, t:t + 1], tx2rb, NT)
            for t in range(PT):
                emit_gram(cxx, t, txt, txt[:, 0:P], tx2c[:, t:t + 1],
                          tx2rb[:, 0:P], P)

            # ---- Schur-recursion inverse with Newton-Schulz leaves ----
            minv = big.tile([P, PT, N], FP)

            def emit_ns(a_blk, o_blk):
                rs = nsp.tile([P, 1], FP, tag="ns_rs")
                nc.vector.tensor_reduce(rs[:], a_blk, mybir.AxisListType.X,
                                        ADD, apply_absolute_value=True)
                mx = nsp.tile([P, 1], FP, tag="ns_mx")
                nc.gpsimd.partition_all_reduce(mx[:], rs[:], 128,
                                               bass_isa.ReduceOp.max)
                cc = nsp.tile([P, 1], FP, tag="ns_cc")
                nc.vector.reciprocal(cc[:], mx[:])
                # X0 = cc * (2I - cc*A)
                xcur = nsp.tile([P, P], FP, tag="ns_x")
                t0 = nsp.tile([P, P], FP, tag="ns_t0")
                nc.vector.tensor_scalar_mul(t0[:], a_blk, cc[:])
                nc.vector.scalar_tensor_tensor(t0[:], t0[:], -1.0, ident2[:],
                                               op0=MULT, op1=ADD)
                nc.vector.tensor_scalar_mul(xcur[:], t0[:], cc[:])
                for it in range(NS_ITERS - 1):
                    psp = ps128.tile([P, P], FP, tag="mm128")
                    nc.tensor.matmul(psp[:], a_blk, xcur[:],
                                     start=True, stop=True)
                    g = nsp.tile([P, P], FP, tag="ns_g")
                    nc.vector.scalar_tensor_tensor(g[:], psp[:], -1.0,
                                                   ident2[:], op0=MULT,
                                                   op1=ADD)
                    psx = ps128.tile([P, P], FP, tag="mm128")
                    nc.tensor.matmul(psx[:], xcur[:], g[:],
                                     start=True, stop=True)
                    if it == NS_ITERS - 2:
                        nc.vector.tensor_copy(o_blk, psx[:])
                    else:
                        xnxt = nsp.tile([P, P], FP, tag="ns_x")
                        nc.vector.tensor_copy(xnxt[:], psx[:])
                        xcur = xnxt

            def emit_transpose_inplace(G, ht):
                # G <- G^T, block-pair swaps staged through PSUM
                for i in range(ht):
                    for j in range(i, ht):
                        ps1 = ps128.tile([P, P], FP, tag="mm128")
                        nc.tensor.transpose(ps1[:], G.blk(i, j), ident[:])
                        if i == j:
                            nc.scalar.copy(G.blk(i, i), ps1[:])
                        else:
                            ps2 = ps128.tile([P, P], FP, tag="mm128")
                            nc.tensor.transpose(ps2[:], G.blk(j, i), ident[:])
                            nc.scalar.copy(G.blk(j, i), ps1[:])
                            nc.scalar.copy(G.blk(i, j), ps2[:])

            def emit_inv(n, mv, ov):
                """ov <- mv^-1.  Ai and Si live directly in ov's quadrants."""
                if n == P:
                    emit_ns(mv.blk(0, 0), ov.blk(0, 0))
                    return
                h = n // 2
                ht = h // P
                A, B, Dd = mv.sub(0, 0), mv.sub(0, h), mv.sub(h, h)
                g_t = work.tile([P, ht, h], FP, tag=f"g{n}")
                sh_t = work.tile([P, ht, h], FP, tag=f"sh{n}")
                G = MV(g_t)
                S = MV(sh_t)
                Ai = ov.sub(0, 0)
                Si = ov.sub(h, h)

                emit_inv(h, A, Ai)
                # G = Ai @ B
                mm512(Ai, B,
                      lambda ps, mt, c0, w: nc.scalar.copy(G.row(mt, c0, w),
                                                           ps),
                      ht, h, ht)
                # iterative refinement: G += Ai (B - A G).  Keeps the Schur
                # complement accurate even though each NS inverse only
                # reaches its own kappa*eps floor; without it the ||B||^2
                # amplification makes the top-level S indefinite in fp32.
                Rv = MV(sh_t)  # S's storage is free until S is formed
                for _ in range(2 if n >= 512 else 1):
                    mm512(A, G,
                          lambda ps, mt, c0, w: nc.vector.scalar_tensor_tensor(
                              Rv.row(mt, c0, w), ps, -1.0, B.row(mt, c0, w),
                              op0=MULT, op1=ADD),
                          ht, h, ht)
                    mm512(Ai, Rv,
                          lambda ps, mt, c0, w: nc.vector.tensor_add(
                              G.row(mt, c0, w), ps, G.row(mt, c0, w)),
                          ht, h, ht)
                # S = D - B^T G
                mm512(B, G,
                      lambda ps, mt, c0, w: nc.vector.scalar_tensor_tensor(
                          S.row(mt, c0, w), ps, -1.0, Dd.row(mt, c0, w),
                          op0=MULT, op1=ADD),
                      ht, h, ht)
                emit_inv(h, S, Si)
                # G <- G^T  (S dead -> its tile becomes H's storage)
                emit_transpose_inplace(G, ht)
                Gt = G
                H = MV(sh_t)
                # H = Si @ G^T
                mm512(Si, Gt,
                      lambda ps, mt, c0, w: nc.scalar.copy(H.row(mt, c0, w),
                                                           ps),
                      ht, h, ht)
                # TL = Ai + G @ H   (in place: Ai lives in ov(0,0))
                mm512(Gt, H,
                      lambda ps, mt, c0, w: nc.vector.tensor_add(
                          Ai.row(mt, c0, w), ps, Ai.row(mt, c0, w)),
                      ht, h, ht)
                # TR = -(G @ Si) ; BL = -H
                mm512(Gt, Si,
                      lambda ps, mt, c0, w: nc.vector.tensor_scalar_mul(
                          ov.sub(0, h).row(mt, c0, w), ps, -1.0),
                      ht, h, ht)
                for mt in range(ht):
                    nc.scalar.mul(ov.sub(h, 0).row(mt, 0, h),
                                  H.row(mt, 0, h), -1.0)

            if dump_minv:
                for t in range(PT):
                    nc.sync.dma_start(o_dbg[t * P:(t + 1) * P, :],
                                      mmat[:, t, :])

            emit_inv(N, MV(mmat), MV(minv))

            if dump_minv:
                for t in range(PT):
                    nc.sync.dma_start(o_dbg[N + t * P:N + (t + 1) * P, :],
                                      minv[:, t, :])

            # ---- 8-rank AllGather of the inverses (pairs contribute dup) ----
            for t in range(PT):
                nc.sync.dma_start(ag_in.ap()[t * P:(t + 1) * P, :],
                                  minv[:, t, :])
            nc.gpsimd.collective_compute(
                "AllGather", mybir.AluOpType.bypass,
                replica_groups=[list(range(NC))],
                ins=[ag_in.ap().opt()],
                outs=[ag_out.ap().opt()],
            )

            # ---- CxZ = [Cx[:, 0:128] | Z1] ----
            cxz = work.tile([P, PT, UW], FP)
            for t in range(PT):
                nc.vector.tensor_copy(cxz[:, t, 0:P], cx[:, t, 0:P])
                nc.vector.tensor_copy(cxz[:, t, P:UW], z1[:, t, :])

            # ---- U_k = Minv_k @ CxZ ;  T_k = Cx^T @ U_k[:, :128] ----
            # Minv_k streamed from the gather buffer in two half strips
            # (contiguous 2 MiB DMAs); combined tile [*, k*PT+mt, 0:UW]=U,
            # [*, k*PT+mt, UW:UTW]=T.
            ut = big.tile([P, L * PT, UTW], FP)
            for k in range(L):
                for q in range(4):
                    strip = mstr.tile([P, 2, N], FP, tag="mstrip")
                    src = ag_out.ap()[k * N + q * 256:k * N + (q + 1) * 256,
                                      :]
                    nc.sync.dma_start(
                        strip[:],
                        src.rearrange("(a p) c -> p a c", p=P))
                    for mt in range(PT):
                        ps = ps132.tile([P, UW], FP, tag="umm")
                        for kl in range(2):
                            nc.tensor.matmul(
                                ps[:], strip[:, kl, mt * P:(mt + 1) * P],
                                cxz[:, q * 2 + kl, :],
                                start=(kl == 0), stop=(kl == 1))
                        if q == 0:
                            nc.scalar.copy(ut[:, k * PT + mt, 0:UW], ps[:])
                        else:
                            nc.vector.tensor_add(ut[:, k * PT + mt, 0:UW],
                                                 ps[:],
                                                 ut[:, k * PT + mt, 0:UW])
                for mt in range(PT):
                    ps = ps128.tile([P, P], FP, tag="mm128")
                    for kt in range(PT):
                        nc.tensor.matmul(ps[:],
                                         cx[:, kt, mt * P:(mt + 1) * P],
                                         ut[:, k * PT + kt, 0:P],
                                         start=(kt == 0), stop=(kt == PT - 1))
                    nc.scalar.copy(ut[:, k * PT + mt, UW:UTW], ps[:])

            # ---- V = Z2 @ (W^T task_K) ;  mean = Cx[:, :128]^T @ V ----
            v = work.tile([P, PT, L], FP)
            for i in range(L):
                for k in range(L):
                    z2k = ut[:, k * PT:(k + 1) * PT, P + k]
                    if k == 0:
                        nc.vector.tensor_scalar_mul(v[:, :, i], z2k,
                                                    cs(C_WT2 + 4 * k + i))
                    else:
                        nc.vector.scalar_tensor_tensor(
                            v[:, :, i], z2k, cs(C_WT2 + 4 * k + i), v[:, :, i],
                            op0=MULT, op1=ADD)
            psm = ps128.tile([P, L], FP, tag="mm128")
            for kt in range(PT):
                nc.tensor.matmul(psm[:], cx[:, kt, 0:P], v[:, kt, :],
                                 start=(kt == 0), stop=(kt == PT - 1))
            meansb = work.tile([P, L], FP)
            nc.vector.tensor_copy(meansb[:], psm[:])
            nc.sync.dma_start(o_mean, meansb[:])

            # ---- fvar stripes: only task blocks i <= j (host mirrors) ----
            for i in range(L):
                for t in range(PT):
                    wj = (L - i) * P
                    acc = accp.tile([P, (L) * P], FP, tag="acc")
                    for j in range(i, L):
                        sub = acc[:, (j - i) * P:(j - i + 1) * P]
                        nc.scalar.mul(sub, cxx[:, t, :], cs(C_TK + 4 * i + j))
                        for k in range(L):
                            nc.vector.scalar_tensor_tensor(
                                sub, ut[:, k * PT + t, UW:UTW],
                                cs(C_SC + 16 * i + 4 * j + k), sub,
                                op0=MULT, op1=ADD)
                    nc.sync.dma_start(
                        o_fvar[i * NT + t * P:i * NT + (t + 1) * P,
                               i * P:i * P + wj], acc[:, 0:wj])

    nc.compile()
    return nc


_NC_CACHE = [None]


def _get_program():
    if _NC_CACHE[0] is None:
        _NC_CACHE[0] = _build_program()
    return _NC_CACHE[0]


def _host_prep(X, test_X, Y, log_noise, covar_factor, log_var, log_lengthscale):
    X = np.asarray(X, f32)
    test_X = np.asarray(test_X, f32)
    Y = np.asarray(Y, f32)
    log_noise = np.asarray(log_noise, f32)
    covar_factor = np.asarray(covar_factor, f32)
    log_var = np.asarray(log_var, f32)
    ls = float(np.asarray(log_lengthscale, f32))

    task_K = (covar_factor @ covar_factor.T
              + np.diag(np.exp(log_var))).astype(f32)
    d = np.exp(log_noise).astype(f32)
    dih = (1.0 / np.sqrt(d)).astype(f32)
    lam, Pm = np.linalg.eigh(
        (dih[:, None] * task_K * dih[None, :]).astype(np.float64))
    lam = lam.astype(f32)
    Pm = Pm.astype(f32)
    W = (dih[:, None] * Pm).astype(f32)
    Smat = (task_K @ W).astype(f32)
    Wt2 = (W.T @ task_K).astype(f32)
    Z1 = (Y @ W).astype(f32)
    expsc = f32(-0.5 * np.exp(-2.0 * ls))

    x2 = np.sum(X * X, axis=1).astype(f32)
    tx2 = np.sum(test_X * test_X, axis=1).astype(f32)

    in_maps = []
    for c in range(NC):
        k = c % 4
        perm = np.roll(np.arange(NT), -P * c)
        trot = test_X[perm]
        tx2rot = tx2[perm]
        cstv = np.zeros((CN,), f32)
        cstv[C_LAM] = lam[k]
        cstv[C_EXPSC] = expsc
        for i in range(L):
            for j in range(L):
                cstv[C_TK + 4 * i + j] = task_K[i, j]
                for kk in range(L):
                    cstv[C_SC + 16 * i + 4 * j + kk] = \
                        -Smat[i, kk] * Smat[j, kk]
        for kk in range(L):
            for i in range(L):
                cstv[C_WT2 + 4 * kk + i] = Wt2[kk, i]
        in_maps.append({
            "xt": np.ascontiguousarray(X.T),
            "txt": np.ascontiguousarray(trot.T),
            "x2c": np.ascontiguousarray(expsc * x2.reshape(PT, P).T),
            "x2r": x2.reshape(1, N).copy(),
            "tx2c": np.ascontiguousarray(expsc * tx2rot.reshape(PT, P).T),
            "tx2r": tx2rot.reshape(1, NT).copy(),
            "z1": Z1,
            "cst": np.broadcast_to(cstv, (P, CN)).copy(),
        })
    return in_maps, d


def _stitch(results, d):
    fvar = np.empty((L * NT, L * NT), f32)
    mean = np.empty((NT, L), f32)
    for c in range(NC):
        fp = results[c]["fvar_part"]
        mean[c * P:(c + 1) * P, :] = results[c]["mean_part"]
        for i in range(L):
            blk = np.roll(fp[i * NT:(i + 1) * NT, :], P * c, axis=0)
            for j in range(i, L):
                fvar[i * NT:(i + 1) * NT,
                     j * NT + c * P:j * NT + (c + 1) * P] = \
                    blk[:, j * P:(j + 1) * P]
    for i in range(1, L):
        for j in range(i):
            fvar[i * NT:(i + 1) * NT, j * NT:(j + 1) * NT] = \
                fvar[j * NT:(j + 1) * NT, i * NT:(i + 1) * NT].T
    noise = np.zeros((L * N, L * N), f32)
    noise[np.arange(L * N), np.arange(L * N)] = np.repeat(d, N)
    fmean = mean.T.reshape(-1, 1).copy()
    return fmean, fvar, noise, mean


def kernel(**inputs):
    nc = _get_program()
    in_maps, d = _host_prep(**inputs)
    res = bass_utils.run_bass_kernel_spmd(nc, in_maps,
                                          core_ids=list(range(NC)))
    return _stitch(res.results, d)


# revision 21
# speedup vs baseline: 1.2890x; 1.2370x over previous
"""Multitask exact GP posterior on 8 Trainium2 NeuronCores.

Math: with task_K = B B^T + diag(exp(log_var)) (4x4), D = diag(exp(log_noise)),
C = rbf(X,X), the full covariance K = kron(task_K, C) + kron(D, I_N) decouples
through the 4x4 eigendecomposition D^-1/2 task_K D^-1/2 = P diag(lam) P^T into
four independent 1024x1024 SPD solves (I + lam_k C)^-1.  All outputs are then:
  W  = D^-1/2 P,  S = task_K W,  Minv_k = (I + lam_k C)^-1
  T_k = Cx^T Minv_k Cx
  fvar[(i,n),(j,m)] = task_K[i,j]*Cxx[n,m] - sum_k S[i,k]S[j,k] T_k[n,m]
  Z2[:,k] = Minv_k (Y W)[:,k]
  mean_mat = Cx^T (Z2 W^T task_K)   (out4; fmean = vec of its transpose)
  noise = kron(D, I_N)  (diagonal; assembled on host, zero FLOPs)

Device (SPMD, uniform program, per-core data):
  core c in 0..7 handles shift k = c%4 (pairs duplicate) and the 128-column
  test block starting at 128*c.  test_X is rotated by 128*c per core so the
  program stays uniform; the host un-rotates rows when stitching fvar.
  Each core: rbf grams (M = I + lam_k C formed in place, Cx full, Cxx
  128-col slice) -> Schur-recursion inverse (Newton-Schulz 128x128 leaves,
  per-level iterative refinement of G = A^-1 B) -> 8-rank AllGather of the
  inverses -> U_k = Minv_k [Cx[:, :128] | Z1] -> T_k = Cx^T U_k ->
  fvar upper-triangle task stripes (host mirrors the rest) + mean slice.
"""
import sys

if "/opt/trn_rl_repo" not in sys.path:
    sys.path.insert(0, "/opt/trn_rl_repo")

import numpy as np

import concourse.bass_isa as bass_isa
import concourse.mybir as mybir
import concourse.tile as tile
from concourse import bacc, bass_utils
from concourse.masks import make_identity

f32 = np.float32
FP = mybir.dt.float32
N = 1024          # train points
NT = 1024         # test points
D_IN = 8          # input dim
L = 4             # tasks
NC = 8            # cores
PT = 8            # 128-row tiles per 1024
P = 128
NS_ITERS = 8
UW = 132          # U columns: 128 test cols + 4 Z1 cols
UTW = UW + P      # combined U|T row width

# consts tensor column layout (per-core [128, CN] fp32, value broadcast
# down the partition axis so any column slices as a [128,1] scalar AP)
C_LAM = 0         # lam_k for this core
C_EXPSC = 1       # -0.5 / lengthscale^2
C_TK = 2          # 16: task_K[i,j] at 2 + 4*i + j
C_SC = 18         # 64: -S[i,k]*S[j,k] at 18 + 16*i + 4*j + k
C_WT2 = 82        # 16: (W^T task_K)[k,i] at 82 + 4*k + i
C_HSEL = 98       # 1.0 if this core contributes the TOP half of Minv
C_HSEL1 = 99      # 1.0 - C_HSEL
CN = 100

MULT = mybir.AluOpType.mult
ADD = mybir.AluOpType.add


class MV:
    """Block view of a [128, ntiles, ncols] SBUF tile holding a matrix whose
    element (rt*128+p, c) lives at ap[p, r0+rt, c0+c]."""

    def __init__(self, ap, r0=0, c0=0):
        self.ap = ap
        self.r0 = r0
        self.c0 = c0

    def sub(self, r, c):
        return MV(self.ap, self.r0 + r // P, self.c0 + c)

    def blk(self, i, j):
        return self.ap[:, self.r0 + i, self.c0 + j * P:self.c0 + (j + 1) * P]

    def row(self, i, c_start, w):
        return self.ap[:, self.r0 + i, self.c0 + c_start:self.c0 + c_start + w]


def _build_program(dump_minv=False):
    nc = bacc.Bacc("TRN2", target_bir_lowering=False, debug=False,
                   num_devices=NC)

    # I/O
    i_xt = nc.dram_tensor("xt", [D_IN, N], FP, kind="ExternalInput").ap()
    i_txt = nc.dram_tensor("txt", [D_IN, NT], FP, kind="ExternalInput").ap()
    i_x2c = nc.dram_tensor("x2c", [P, PT], FP, kind="ExternalInput").ap()
    i_x2r = nc.dram_tensor("x2r", [1, N], FP, kind="ExternalInput").ap()
    i_tx2c = nc.dram_tensor("tx2c", [P, PT], FP, kind="ExternalInput").ap()
    i_tx2r = nc.dram_tensor("tx2r", [1, NT], FP, kind="ExternalInput").ap()
    i_z1 = nc.dram_tensor("z1", [N, L], FP, kind="ExternalInput").ap()
    i_cst = nc.dram_tensor("cst", [P, CN], FP, kind="ExternalInput").ap()
    o_fvar = nc.dram_tensor("fvar_part", [L * NT, L * P], FP,
                            kind="ExternalOutput").ap()
    o_mean = nc.dram_tensor("mean_part", [P, L], FP, kind="ExternalOutput").ap()

    ag_in = nc.dram_tensor("ag_in", [N // 2, N], FP)
    ag_out = nc.dram_tensor("ag_out", [NC * (N // 2), N], FP)
    o_dbg = None
    if dump_minv:
        o_dbg = nc.dram_tensor("dbg_mm", [2 * N, N], FP,
                               kind="ExternalOutput").ap()

    with tile.TileContext(nc) as tc:
        with (
            tc.tile_pool(name="const", bufs=1) as consts,
            tc.tile_pool(name="big", bufs=1) as big,
            tc.tile_pool(name="work", bufs=1) as work,
            tc.tile_pool(name="ns", bufs=2) as nsp,
            tc.tile_pool(name="mstr", bufs=2) as mstr,
            tc.tile_pool(name="ps512", bufs=2, space="PSUM") as ps512,
            tc.tile_pool(name="ps132", bufs=2, space="PSUM") as ps132,
            tc.tile_pool(name="ps128", bufs=4, space="PSUM") as ps128,
        ):
            def mm512(lhsT, rhs, evict, m_tiles, n_cols, k_tiles):
                """out[mt, c0:c0+w] = sum_kt lhsT.blk(kt,mt)^T @ rhs.row(...)"""
                for mt in range(m_tiles):
                    for c0 in range(0, n_cols, 512):
                        w = min(512, n_cols - c0)
                        ps = ps512.tile([P, 512], FP, tag="s512")
                        for kt in range(k_tiles):
                            nc.tensor.matmul(ps[:, :w], lhsT.blk(kt, mt),
                                             rhs.row(kt, c0, w),
                                             start=(kt == 0),
                                             stop=(kt == k_tiles - 1))
                        evict(ps[:, :w], mt, c0, w)

            # ---- constants ----
            ident = consts.tile([P, P], FP)
            make_identity(nc, ident)
            ident2 = consts.tile([P, P], FP)
            nc.vector.tensor_scalar_mul(ident2[:], ident[:], 2.0)
            cst = consts.tile([P, CN], FP)
            nc.sync.dma_start(cst[:], i_cst)

            def cs(col):
                return cst[:, col:col + 1]

            xt = consts.tile([D_IN, N], FP)
            nc.sync.dma_start(xt[:], i_xt)
            txt = consts.tile([D_IN, NT], FP)
            nc.sync.dma_start(txt[:], i_txt)
            x2c = consts.tile([P, PT], FP)   # pre-scaled by expsc (host)
            nc.sync.dma_start(x2c[:], i_x2c)
            tx2c = consts.tile([P, PT], FP)  # pre-scaled by expsc (host)
            nc.sync.dma_start(tx2c[:], i_tx2c)
            x2r1 = consts.tile([1, N], FP)
            nc.sync.dma_start(x2r1[:], i_x2r)
            tx2r1 = consts.tile([1, NT], FP)
            nc.sync.dma_start(tx2r1[:], i_tx2r)
            x2rb = consts.tile([P, N], FP)
            nc.gpsimd.partition_broadcast(x2rb[:], x2r1[:])
            tx2rb = consts.tile([P, NT], FP)
            nc.gpsimd.partition_broadcast(tx2rb[:], tx2r1[:])
            z1 = consts.tile([P, PT, L], FP)
            for t in range(PT):
                nc.sync.dma_start(z1[:, t, :], i_z1[t * P:(t + 1) * P, :])

            # ---- rbf grams;  M = lam * rbf(X,X) + I formed in place ----
            mmat = big.tile([P, PT, N], FP)     # M for this core's shift
            cx = big.tile([P, PT, NT], FP)      # Cx = rbf(X, test_rot)
            cxx = work.tile([P, PT, P], FP)     # Cxx[:, first 128 rot cols]

            def emit_gram(dst, t, lhs_cols, rhs_all, col_bias, row_b, n_cols):
                # dst[t] = exp(expsc*(-2*lhs^T rhs + rowv) + col_bias)
                for c0 in range(0, n_cols, 512):
                    w = min(512, n_cols - c0)
                    ps = ps512.tile([P, 512], FP, tag="s512")
                    nc.tensor.matmul(ps[:, :w],
                                     lhs_cols[:, t * P:(t + 1) * P],
                                     rhs_all[:, c0:c0 + w],
                                     start=True, stop=True)
                    d2 = dst[:, t, c0:c0 + w]
                    nc.vector.scalar_tensor_tensor(
                        d2, ps[:, :w], -2.0, row_b[:, c0:c0 + w],
                        op0=MULT, op1=ADD)
                    nc.scalar.activation(d2, d2,
                                         mybir.ActivationFunctionType.Exp,
                                         scale=cs(C_EXPSC), bias=col_bias)

            for t in range(PT):
                emit_gram(mmat, t, xt, xt, x2c[:, t:t + 1], x2rb, N)
                nc.vector.tensor_scalar_mul(mmat[:, t, :], mmat[:, t, :],
                                            cs(C_LAM))
                nc.vector.tensor_add(mmat[:, t, t * P:(t + 1) * P],
                                     mmat[:, t, t * P:(t + 1) * P], ident[:])
            for t in range(PT):
                emit_gram(cx, t, xt, txt, x2c[:, t:t + 1], tx2rb, NT)
            for t in range(PT):
                emit_gram(cxx, t, txt, txt[:, 0:P], tx2c[:, t:t + 1],
                          tx2rb[:, 0:P], P)

            # ---- Schur-recursion inverse with Newton-Schulz leaves ----
            minv = big.tile([P, PT, N], FP)

            def emit_ns(a_blk, o_blk):
                rs = nsp.tile([P, 1], FP, tag="ns_rs")
                nc.vector.tensor_reduce(rs[:], a_blk, mybir.AxisListType.X,
                                        ADD, apply_absolute_value=True)
                mx = nsp.tile([P, 1], FP, tag="ns_mx")
                nc.gpsimd.partition_all_reduce(mx[:], rs[:], 128,
                                               bass_isa.ReduceOp.max)
                cc = nsp.tile([P, 1], FP, tag="ns_cc")
                nc.vector.reciprocal(cc[:], mx[:])
                # X0 = cc * (2I - cc*A)
                xcur = nsp.tile([P, P], FP, tag="ns_x")
                t0 = nsp.tile([P, P], FP, tag="ns_t0")
                nc.vector.tensor_scalar_mul(t0[:], a_blk, cc[:])
                nc.vector.scalar_tensor_tensor(t0[:], t0[:], -1.0, ident2[:],
                                               op0=MULT, op1=ADD)
                nc.vector.tensor_scalar_mul(xcur[:], t0[:], cc[:])
                for it in range(NS_ITERS - 1):
                    psp = ps128.tile([P, P], FP, tag="mm128")
                    nc.tensor.matmul(psp[:], a_blk, xcur[:],
                                     start=True, stop=True)
                    g = nsp.tile([P, P], FP, tag="ns_g")
                    nc.vector.scalar_tensor_tensor(g[:], psp[:], -1.0,
                                                   ident2[:], op0=MULT,
                                                   op1=ADD)
                    psx = ps128.tile([P, P], FP, tag="mm128")
                    nc.tensor.matmul(psx[:], xcur[:], g[:],
                                     start=True, stop=True)
                    if it == NS_ITERS - 2:
                        nc.vector.tensor_copy(o_blk, psx[:])
                    else:
                        xnxt = nsp.tile([P, P], FP, tag="ns_x")
                        nc.vector.tensor_copy(xnxt[:], psx[:])
                        xcur = xnxt

            def emit_transpose_inplace(G, ht):
                # G <- G^T, block-pair swaps staged through PSUM
                for i in range(ht):
                    for j in range(i, ht):
                        ps1 = ps128.tile([P, P], FP, tag="mm128")
                        nc.tensor.transpose(ps1[:], G.blk(i, j), ident[:])
                        if i == j:
                            nc.scalar.copy(G.blk(i, i), ps1[:])
                        else:
                            ps2 = ps128.tile([P, P], FP, tag="mm128")
                            nc.tensor.transpose(ps2[:], G.blk(j, i), ident[:])
                            nc.scalar.copy(G.blk(j, i), ps1[:])
                            nc.scalar.copy(G.blk(i, j), ps2[:])

            def emit_inv(n, mv, ov):
                """ov <- mv^-1.  Ai and Si live directly in ov's quadrants."""
                if n == P:
                    emit_ns(mv.blk(0, 0), ov.blk(0, 0))
                    return
                h = n // 2
                ht = h // P
                A, B, Dd = mv.sub(0, 0), mv.sub(0, h), mv.sub(h, h)
                g_t = work.tile([P, ht, h], FP, tag=f"g{n}")
                sh_t = work.tile([P, ht, h], FP, tag=f"sh{n}")
                G = MV(g_t)
                S = MV(sh_t)
                Ai = ov.sub(0, 0)
                Si = ov.sub(h, h)

                emit_inv(h, A, Ai)
                # G = Ai @ B
                mm512(Ai, B,
                      lambda ps, mt, c0, w: nc.scalar.copy(G.row(mt, c0, w),
                                                           ps),
                      ht, h, ht)
                # iterative refinement: G += Ai (B - A G).  Keeps the Schur
                # complement accurate even though each NS inverse only
                # reaches its own kappa*eps floor; without it the ||B||^2
                # amplification makes the top-level S indefinite in fp32.
                Rv = MV(sh_t)  # S's storage is free until S is formed
                for _ in range(2 if n >= 512 else 1):
                    mm512(A, G,
                          lambda ps, mt, c0, w: nc.vector.scalar_tensor_tensor(
                              Rv.row(mt, c0, w), ps, -1.0, B.row(mt, c0, w),
                              op0=MULT, op1=ADD),
                          ht, h, ht)
                    mm512(Ai, Rv,
                          lambda ps, mt, c0, w: nc.vector.tensor_add(
                              G.row(mt, c0, w), ps, G.row(mt, c0, w)),
                          ht, h, ht)
                # S = D - B^T G
                mm512(B, G,
                      lambda ps, mt, c0, w: nc.vector.scalar_tensor_tensor(
                          S.row(mt, c0, w), ps, -1.0, Dd.row(mt, c0, w),
                          op0=MULT, op1=ADD),
                      ht, h, ht)
                emit_inv(h, S, Si)
                # G <- G^T  (S dead -> its tile becomes H's storage)
                emit_transpose_inplace(G, ht)
                Gt = G
                H = MV(sh_t)
                # H = Si @ G^T
                mm512(Si, Gt,
                      lambda ps, mt, c0, w: nc.scalar.copy(H.row(mt, c0, w),
                                                           ps),
                      ht, h, ht)
                # TL = Ai + G @ H   (in place: Ai lives in ov(0,0))
                mm512(Gt, H,
                      lambda ps, mt, c0, w: nc.vector.tensor_add(
                          Ai.row(mt, c0, w), ps, Ai.row(mt, c0, w)),
                      ht, h, ht)
                # TR = -(G @ Si) ; BL = -H
                mm512(Gt, Si,
                      lambda ps, mt, c0, w: nc.vector.tensor_scalar_mul(
                          ov.sub(0, h).row(mt, c0, w), ps, -1.0),
                      ht, h, ht)
                for mt in range(ht):
                    nc.scalar.mul(ov.sub(h, 0).row(mt, 0, h),
                                  H.row(mt, 0, h), -1.0)

            if dump_minv:
                for t in range(PT):
                    nc.sync.dma_start(o_dbg[t * P:(t + 1) * P, :],
                                      mmat[:, t, :])

            emit_inv(N, MV(mmat), MV(minv))

            if dump_minv:
                for t in range(PT):
                    nc.sync.dma_start(o_dbg[N + t * P:N + (t + 1) * P, :],
                                      minv[:, t, :])

            # ---- 8-rank AllGather; core c contributes the top (c<4) or
            # bottom (c>=4) half of its Minv, selected by a data-driven
            # blend so the program stays uniform.  ag_out slot k holds
            # Minv_k rows [0:512), slot k+4 holds rows [512:1024).
            for a in range(2):
                stage = mstr.tile([P, 2, N], FP, tag="mstrip")
                nc.vector.tensor_scalar_mul(stage[:],
                                            minv[:, 4 + 2 * a:6 + 2 * a, :],
                                            cs(C_HSEL1))
                nc.vector.scalar_tensor_tensor(
                    stage[:], minv[:, 2 * a:2 * a + 2, :], cs(C_HSEL),
                    stage[:], op0=MULT, op1=ADD)
                nc.sync.dma_start(
                    ag_in.ap()[a * 256:(a + 1) * 256, :]
                    .rearrange("(t p) c -> p t c", p=P),
                    stage[:])
            nc.gpsimd.collective_compute(
                "AllGather", mybir.AluOpType.bypass,
                replica_groups=[list(range(NC))],
                ins=[ag_in.ap().opt()],
                outs=[ag_out.ap().opt()],
            )

            # ---- CxZ = [Cx[:, 0:128] | Z1] ----
            cxz = work.tile([P, PT, UW], FP)
            for t in range(PT):
                nc.vector.tensor_copy(cxz[:, t, 0:P], cx[:, t, 0:P])
                nc.vector.tensor_copy(cxz[:, t, P:UW], z1[:, t, :])

            # ---- U_k = Minv_k @ CxZ ;  T_k = Cx^T @ U_k[:, :128] ----
            # Minv_k streamed from the gather buffer in two half strips
            # (contiguous 2 MiB DMAs); combined tile [*, k*PT+mt, 0:UW]=U,
            # [*, k*PT+mt, UW:UTW]=T.
            ut = big.tile([P, L * PT, UTW], FP)
            for k in range(L):
                for q in range(4):
                    strip = mstr.tile([P, 2, N], FP, tag="mstrip")
                    if q < 2:
                        r0 = k * 512 + q * 256
                    else:
                        r0 = (k + 4) * 512 + (q - 2) * 256
                    src = ag_out.ap()[r0:r0 + 256, :]
                    nc.sync.dma_start(
                        strip[:],
                        src.rearrange("(a p) c -> p a c", p=P))
                    for mt in range(PT):
                        ps = ps132.tile([P, UW], FP, tag="umm")
                        for kl in range(2):
                            nc.tensor.matmul(
                                ps[:], strip[:, kl, mt * P:(mt + 1) * P],
                                cxz[:, q * 2 + kl, :],
                                start=(kl == 0), stop=(kl == 1))
                        if q == 0:
                            nc.scalar.copy(ut[:, k * PT + mt, 0:UW], ps[:])
                        else:
                            nc.vector.tensor_add(ut[:, k * PT + mt, 0:UW],
                                                 ps[:],
                                                 ut[:, k * PT + mt, 0:UW])
                for mt in range(PT):
                    ps = ps128.tile([P, P], FP, tag="mm128")
                    for kt in range(PT):
                        nc.tensor.matmul(ps[:],
                                         cx[:, kt, mt * P:(mt + 1) * P],
                                         ut[:, k * PT + kt, 0:P],
                                         start=(kt == 0), stop=(kt == PT - 1))
                    nc.scalar.copy(ut[:, k * PT + mt, UW:UTW], ps[:])

            # ---- V = Z2 @ (W^T task_K) ;  mean = Cx[:, :128]^T @ V ----
            v = work.tile([P, PT, L], FP)
            for i in range(L):
                for k in range(L):
                    z2k = ut[:, k * PT:(k + 1) * PT, P + k]
                    if k == 0:
                        nc.vector.tensor_scalar_mul(v[:, :, i], z2k,
                                                    cs(C_WT2 + 4 * k + i))
                    else:
                        nc.vector.scalar_tensor_tensor(
                            v[:, :, i], z2k, cs(C_WT2 + 4 * k + i), v[:, :, i],
                            op0=MULT, op1=ADD)
            psm = ps128.tile([P, L], FP, tag="mm128")
            for kt in range(PT):
                nc.tensor.matmul(psm[:], cx[:, kt, 0:P], v[:, kt, :],
                                 start=(kt == 0), stop=(kt == PT - 1))
            meansb = work.tile([P, L], FP)
            nc.vector.tensor_copy(meansb[:], psm[:])
            nc.sync.dma_start(o_mean, meansb[:])

            # ---- fvar stripes: only task blocks i <= j (host mirrors).
            # Two row-tiles per op (wide APs) to halve instruction count;
            # acc tiles reuse the dead top-level Schur scratch slots.
            for i in range(L):
                for tp in range(PT // 2):
                    wj = (L - i) * P
                    tag = "g1024" if tp % 2 == 0 else "sh1024"
                    acc = work.tile([P, 2, L * P], FP, tag=tag)
                    for j in range(i, L):
                        sub = acc[:, :, (j - i) * P:(j - i + 1) * P]
                        nc.scalar.mul(sub, cxx[:, 2 * tp:2 * tp + 2, :],
                                      cs(C_TK + 4 * i + j))
                        for k in range(L):
                            nc.vector.scalar_tensor_tensor(
                                sub,
                                ut[:, k * PT + 2 * tp:k * PT + 2 * tp + 2,
                                   UW:UTW],
                                cs(C_SC + 16 * i + 4 * j + k), sub,
                                op0=MULT, op1=ADD)
                    nc.sync.dma_start(
                        o_fvar[i * NT + 2 * tp * P:i * NT + (2 * tp + 2) * P,
                               i * P:i * P + wj]
                        .rearrange("(t p) c -> p t c", p=P),
                        acc[:, :, 0:wj])

    nc.compile()
    return nc


_NC_CACHE = [None]


def _get_program():
    if _NC_CACHE[0] is None:
        _NC_CACHE[0] = _build_program()
    return _NC_CACHE[0]


def _host_prep(X, test_X, Y, log_noise, covar_factor, log_var, log_lengthscale):
    X = np.asarray(X, f32)
    test_X = np.asarray(test_X, f32)
    Y = np.asarray(Y, f32)
    log_noise = np.asarray(log_noise, f32)
    covar_factor = np.asarray(covar_factor, f32)
    log_var = np.asarray(log_var, f32)
    ls = float(np.asarray(log_lengthscale, f32))

    task_K = (covar_factor @ covar_factor.T
              + np.diag(np.exp(log_var))).astype(f32)
    d = np.exp(log_noise).astype(f32)
    dih = (1.0 / np.sqrt(d)).astype(f32)
    lam, Pm = np.linalg.eigh(
        (dih[:, None] * task_K * dih[None, :]).astype(np.float64))
    lam = lam.astype(f32)
    Pm = Pm.astype(f32)
    W = (dih[:, None] * Pm).astype(f32)
    Smat = (task_K @ W).astype(f32)
    Wt2 = (W.T @ task_K).astype(f32)
    Z1 = (Y @ W).astype(f32)
    expsc = f32(-0.5 * np.exp(-2.0 * ls))

    x2 = np.sum(X * X, axis=1).astype(f32)
    tx2 = np.sum(test_X * test_X, axis=1).astype(f32)

    in_maps = []
    for c in range(NC):
        k = c % 4
        perm = np.roll(np.arange(NT), -P * c)
        trot = test_X[perm]
        tx2rot = tx2[perm]
        cstv = np.zeros((CN,), f32)
        cstv[C_LAM] = lam[k]
        cstv[C_EXPSC] = expsc
        cstv[C_HSEL] = 1.0 if c < 4 else 0.0
        cstv[C_HSEL1] = 0.0 if c < 4 else 1.0
        for i in range(L):
            for j in range(L):
                cstv[C_TK + 4 * i + j] = task_K[i, j]
                for kk in range(L):
                    cstv[C_SC + 16 * i + 4 * j + kk] = \
                        -Smat[i, kk] * Smat[j, kk]
        for kk in range(L):
            for i in range(L):
                cstv[C_WT2 + 4 * kk + i] = Wt2[kk, i]
        in_maps.append({
            "xt": np.ascontiguousarray(X.T),
            "txt": np.ascontiguousarray(trot.T),
            "x2c": np.ascontiguousarray(expsc * x2.reshape(PT, P).T),
            "x2r": x2.reshape(1, N).copy(),
            "tx2c": np.ascontiguousarray(expsc * tx2rot.reshape(PT, P).T),
            "tx2r": tx2rot.reshape(1, NT).copy(),
            "z1": Z1,
            "cst": np.broadcast_to(cstv, (P, CN)).copy(),
        })
    return in_maps, d


def _stitch(results, d):
    fvar = np.empty((L * NT, L * NT), f32)
    mean = np.empty((NT, L), f32)
    for c in range(NC):
        fp = results[c]["fvar_part"]
        mean[c * P:(c + 1) * P, :] = results[c]["mean_part"]
        for i in range(L):
            blk = np.roll(fp[i * NT:(i + 1) * NT, :], P * c, axis=0)
            for j in range(i, L):
                fvar[i * NT:(i + 1) * NT,
                     j * NT + c * P:j * NT + (c + 1) * P] = \
                    blk[:, j * P:(j + 1) * P]
    for i in range(1, L):
        for j in range(i):
            fvar[i * NT:(i + 1) * NT, j * NT:(j + 1) * NT] = \
                fvar[j * NT:(j + 1) * NT, i * NT:(i + 1) * NT].T
    noise = np.zeros((L * N, L * N), f32)
    noise[np.arange(L * N), np.arange(L * N)] = np.repeat(d, N)
    fmean = mean.T.reshape(-1, 1).copy()
    return fmean, fvar, noise, mean


def kernel(**inputs):
    nc = _get_program()
    in_maps, d = _host_prep(**inputs)
    res = bass_utils.run_bass_kernel_spmd(nc, in_maps,
                                          core_ids=list(range(NC)))
    return _stitch(res.results, d)


# revision 23
# speedup vs baseline: 1.4175x; 1.0997x over previous
"""Multitask exact GP posterior on 8 Trainium2 NeuronCores.

Math: with task_K = B B^T + diag(exp(log_var)) (4x4), D = diag(exp(log_noise)),
C = rbf(X,X), the full covariance K = kron(task_K, C) + kron(D, I_N) decouples
through the 4x4 eigendecomposition D^-1/2 task_K D^-1/2 = P diag(lam) P^T into
four independent 1024x1024 SPD solves (I + lam_k C)^-1.  All outputs are then:
  W  = D^-1/2 P,  S = task_K W,  Minv_k = (I + lam_k C)^-1
  T_k = Cx^T Minv_k Cx
  fvar[(i,n),(j,m)] = task_K[i,j]*Cxx[n,m] - sum_k S[i,k]S[j,k] T_k[n,m]
  Z2[:,k] = Minv_k (Y W)[:,k]
  mean_mat = Cx^T (Z2 W^T task_K)   (out4; fmean = vec of its transpose)
  noise = kron(D, I_N)  (diagonal; assembled on host, zero FLOPs)

Device (SPMD, uniform program, per-core data):
  core c in 0..7 handles shift k = c%4 (pairs duplicate) and the 128-column
  test block starting at 128*c.  test_X is rotated by 128*c per core so the
  program stays uniform; the host un-rotates rows when stitching fvar.
  Each core: rbf grams (M = I + lam_k C formed in place, Cx full, Cxx
  128-col slice) -> Schur-recursion inverse (Newton-Schulz 128x128 leaves,
  per-level iterative refinement of G = A^-1 B) -> 8-rank AllGather of the
  inverses -> U_k = Minv_k [Cx[:, :128] | Z1] -> T_k = Cx^T U_k ->
  fvar upper-triangle task stripes (host mirrors the rest) + mean slice.
"""
import sys

if "/opt/trn_rl_repo" not in sys.path:
    sys.path.insert(0, "/opt/trn_rl_repo")

import numpy as np

import concourse.bass_isa as bass_isa
import concourse.mybir as mybir
import concourse.tile as tile
from concourse import bacc, bass_utils
from concourse.masks import make_identity

f32 = np.float32
FP = mybir.dt.float32
BF = mybir.dt.bfloat16
N = 1024          # train points
NT = 1024         # test points
D_IN = 8          # input dim
L = 4             # tasks
NC = 8            # cores
PT = 8            # 128-row tiles per 1024
P = 128
NS_ITERS = 8
UW = 132          # U columns: 128 test cols + 4 Z1 cols
UTW = UW + P      # combined U|T row width

# consts tensor column layout (per-core [128, CN] fp32, value broadcast
# down the partition axis so any column slices as a [128,1] scalar AP)
C_LAM = 0         # lam_k for this core
C_EXPSC = 1       # -0.5 / lengthscale^2
C_TK = 2          # 16: task_K[i,j] at 2 + 4*i + j
C_SC = 18         # 64: -S[i,k]*S[j,k] at 18 + 16*i + 4*j + k
C_WT2 = 82        # 16: (W^T task_K)[k,i] at 82 + 4*k + i
C_HSEL = 98       # 1.0 if this core contributes the TOP half of Minv
C_HSEL1 = 99      # 1.0 - C_HSEL
CN = 100

MULT = mybir.AluOpType.mult
ADD = mybir.AluOpType.add


class MV:
    """Block view of a [128, ntiles, ncols] SBUF tile holding a matrix whose
    element (rt*128+p, c) lives at ap[p, r0+rt, c0+c]."""

    def __init__(self, ap, r0=0, c0=0):
        self.ap = ap
        self.r0 = r0
        self.c0 = c0

    def sub(self, r, c):
        return MV(self.ap, self.r0 + r // P, self.c0 + c)

    def blk(self, i, j):
        return self.ap[:, self.r0 + i, self.c0 + j * P:self.c0 + (j + 1) * P]

    def row(self, i, c_start, w):
        return self.ap[:, self.r0 + i, self.c0 + c_start:self.c0 + c_start + w]


def _build_program(dump_minv=False):
    nc = bacc.Bacc("TRN2", target_bir_lowering=False, debug=False,
                   num_devices=NC)

    # I/O
    i_xt = nc.dram_tensor("xt", [D_IN, N], FP, kind="ExternalInput").ap()
    i_txt = nc.dram_tensor("txt", [D_IN, NT], FP, kind="ExternalInput").ap()
    i_x2c = nc.dram_tensor("x2c", [P, PT], FP, kind="ExternalInput").ap()
    i_x2r = nc.dram_tensor("x2r", [1, N], FP, kind="ExternalInput").ap()
    i_tx2c = nc.dram_tensor("tx2c", [P, PT], FP, kind="ExternalInput").ap()
    i_tx2r = nc.dram_tensor("tx2r", [1, NT], FP, kind="ExternalInput").ap()
    i_z1 = nc.dram_tensor("z1", [N, L], FP, kind="ExternalInput").ap()
    i_cst = nc.dram_tensor("cst", [P, CN], FP, kind="ExternalInput").ap()
    o_fvar = nc.dram_tensor("fvar_part", [L * NT, L * P], FP,
                            kind="ExternalOutput").ap()
    o_mean = nc.dram_tensor("mean_part", [P, L], FP, kind="ExternalOutput").ap()

    ag_in = nc.dram_tensor("ag_in", [N // 2, N], FP)
    ag_out = nc.dram_tensor("ag_out", [NC * (N // 2), N], FP)
    o_dbg = None
    if dump_minv:
        o_dbg = nc.dram_tensor("dbg_mm", [2 * N, N], FP,
                               kind="ExternalOutput").ap()

    with tile.TileContext(nc) as tc:
        with (
            tc.tile_pool(name="const", bufs=1) as consts,
            tc.tile_pool(name="big", bufs=1) as big,
            tc.tile_pool(name="work", bufs=1) as work,
            tc.tile_pool(name="ns", bufs=2) as nsp,
            tc.tile_pool(name="mstr", bufs=2) as mstr,
            tc.tile_pool(name="ps512", bufs=2, space="PSUM") as ps512,
            tc.tile_pool(name="ps132", bufs=2, space="PSUM") as ps132,
            tc.tile_pool(name="ps128", bufs=4, space="PSUM") as ps128,
        ):
            def mm512(lhsT, rhs, evict, m_tiles, n_cols, k_tiles):
                """out[mt, c0:c0+w] = sum_kt lhsT.blk(kt,mt)^T @ rhs.row(...)"""
                for mt in range(m_tiles):
                    for c0 in range(0, n_cols, 512):
                        w = min(512, n_cols - c0)
                        ps = ps512.tile([P, 512], FP, tag="s512")
                        for kt in range(k_tiles):
                            nc.tensor.matmul(ps[:, :w], lhsT.blk(kt, mt),
                                             rhs.row(kt, c0, w),
                                             start=(kt == 0),
                                             stop=(kt == k_tiles - 1))
                        evict(ps[:, :w], mt, c0, w)

            # ---- constants ----
            ident = consts.tile([P, P], FP)
            make_identity(nc, ident)
            ident2 = consts.tile([P, P], FP)
            nc.vector.tensor_scalar_mul(ident2[:], ident[:], 2.0)
            cst = consts.tile([P, CN], FP)
            nc.sync.dma_start(cst[:], i_cst)

            def cs(col):
                return cst[:, col:col + 1]

            xt = consts.tile([D_IN, N], FP)
            nc.sync.dma_start(xt[:], i_xt)
            txt = consts.tile([D_IN, NT], FP)
            nc.sync.dma_start(txt[:], i_txt)
            x2c = consts.tile([P, PT], FP)   # pre-scaled by expsc (host)
            nc.sync.dma_start(x2c[:], i_x2c)
            tx2c = consts.tile([P, PT], FP)  # pre-scaled by expsc (host)
            nc.sync.dma_start(tx2c[:], i_tx2c)
            x2r1 = consts.tile([1, N], FP)
            nc.sync.dma_start(x2r1[:], i_x2r)
            tx2r1 = consts.tile([1, NT], FP)
            nc.sync.dma_start(tx2r1[:], i_tx2r)
            x2rb = consts.tile([P, N], FP)
            nc.gpsimd.partition_broadcast(x2rb[:], x2r1[:])
            tx2rb = consts.tile([P, NT], FP)
            nc.gpsimd.partition_broadcast(tx2rb[:], tx2r1[:])
            z1 = consts.tile([P, PT, L], FP)
            for t in range(PT):
                nc.sync.dma_start(z1[:, t, :], i_z1[t * P:(t + 1) * P, :])

            # ---- rbf grams;  M = lam * rbf(X,X) + I formed in place ----
            mmat = big.tile([P, PT, N], FP)     # M for this core's shift
            cx = big.tile([P, PT, NT], FP)      # Cx = rbf(X, test_rot)
            cxx = work.tile([P, PT, P], FP)     # Cxx[:, first 128 rot cols]

            def emit_gram(dst, t, lhs_cols, rhs_all, col_bias, row_b, n_cols):
                # dst[t] = exp(expsc*(-2*lhs^T rhs + rowv) + col_bias)
                for c0 in range(0, n_cols, 512):
                    w = min(512, n_cols - c0)
                    ps = ps512.tile([P, 512], FP, tag="s512")
                    nc.tensor.matmul(ps[:, :w],
                                     lhs_cols[:, t * P:(t + 1) * P],
                                     rhs_all[:, c0:c0 + w],
                                     start=True, stop=True)
                    d2 = dst[:, t, c0:c0 + w]
                    nc.vector.scalar_tensor_tensor(
                        d2, ps[:, :w], -2.0, row_b[:, c0:c0 + w],
                        op0=MULT, op1=ADD)
                    nc.scalar.activation(d2, d2,
                                         mybir.ActivationFunctionType.Exp,
                                         scale=cs(C_EXPSC), bias=col_bias)

            for t in range(PT):
                emit_gram(mmat, t, xt, xt, x2c[:, t:t + 1], x2rb, N)
                nc.vector.tensor_scalar_mul(mmat[:, t, :], mmat[:, t, :],
                                            cs(C_LAM))
                nc.vector.tensor_add(mmat[:, t, t * P:(t + 1) * P],
                                     mmat[:, t, t * P:(t + 1) * P], ident[:])
            for t in range(PT):
                emit_gram(cx, t, xt, txt, x2c[:, t:t + 1], tx2rb, NT)
            for t in range(PT):
                emit_gram(cxx, t, txt, txt[:, 0:P], tx2c[:, t:t + 1],
                          tx2rb[:, 0:P], P)

            # ---- Schur-recursion inverse with Newton-Schulz leaves ----
            minv = big.tile([P, PT, N], FP, tag="minv")

            def emit_ns(a_blk, o_blk):
                rs = nsp.tile([P, 1], FP, tag="ns_rs")
                nc.vector.tensor_reduce(rs[:], a_blk, mybir.AxisListType.X,
                                        ADD, apply_absolute_value=True)
                mx = nsp.tile([P, 1], FP, tag="ns_mx")
                nc.gpsimd.partition_all_reduce(mx[:], rs[:], 128,
                                               bass_isa.ReduceOp.max)
                cc = nsp.tile([P, 1], FP, tag="ns_cc")
                nc.vector.reciprocal(cc[:], mx[:])
                # X0 = cc * (2I - cc*A)
                xcur = nsp.tile([P, P], FP, tag="ns_x")
                t0 = nsp.tile([P, P], FP, tag="ns_t0")
                nc.vector.tensor_scalar_mul(t0[:], a_blk, cc[:])
                nc.vector.scalar_tensor_tensor(t0[:], t0[:], -1.0, ident2[:],
                                               op0=MULT, op1=ADD)
                nc.vector.tensor_scalar_mul(xcur[:], t0[:], cc[:])
                for it in range(NS_ITERS - 1):
                    psp = ps128.tile([P, P], FP, tag="mm128")
                    nc.tensor.matmul(psp[:], a_blk, xcur[:],
                                     start=True, stop=True)
                    g = nsp.tile([P, P], FP, tag="ns_g")
                    nc.vector.scalar_tensor_tensor(g[:], psp[:], -1.0,
                                                   ident2[:], op0=MULT,
                                                   op1=ADD)
                    psx = ps128.tile([P, P], FP, tag="mm128")
                    nc.tensor.matmul(psx[:], xcur[:], g[:],
                                     start=True, stop=True)
                    if it == NS_ITERS - 2:
                        nc.vector.tensor_copy(o_blk, psx[:])
                    else:
                        xnxt = nsp.tile([P, P], FP, tag="ns_x")
                        nc.vector.tensor_copy(xnxt[:], psx[:])
                        xcur = xnxt

            def emit_transpose_inplace(G, ht):
                # G <- G^T, block-pair swaps staged through PSUM
                for i in range(ht):
                    for j in range(i, ht):
                        ps1 = ps128.tile([P, P], FP, tag="mm128")
                        nc.tensor.transpose(ps1[:], G.blk(i, j), ident[:])
                        if i == j:
                            nc.scalar.copy(G.blk(i, i), ps1[:])
                        else:
                            ps2 = ps128.tile([P, P], FP, tag="mm128")
                            nc.tensor.transpose(ps2[:], G.blk(j, i), ident[:])
                            nc.scalar.copy(G.blk(j, i), ps1[:])
                            nc.scalar.copy(G.blk(i, j), ps2[:])

            def emit_inv(n, mv, ov):
                """ov <- mv^-1.  Ai and Si live directly in ov's quadrants."""
                if n == P:
                    emit_ns(mv.blk(0, 0), ov.blk(0, 0))
                    return
                h = n // 2
                ht = h // P
                A, B, Dd = mv.sub(0, 0), mv.sub(0, h), mv.sub(h, h)
                g_t = work.tile([P, ht, h], FP, tag=f"g{n}")
                sh_t = work.tile([P, ht, h], FP, tag=f"sh{n}")
                G = MV(g_t)
                S = MV(sh_t)
                Ai = ov.sub(0, 0)
                Si = ov.sub(h, h)

                emit_inv(h, A, Ai)
                # G0 = Ai @ B in bf16 (fast); one refinement pass with an
                # fp32 residual restores full accuracy: G += Ai (B - A G).
                # Without refinement the ||B||^2 amplification of the NS
                # inverse floor makes the top-level S indefinite in fp32;
                # with it, G converges to A^-1 B regardless of G0 quality.
                bai = work.tile([P, ht, h], BF, tag=f"bai{n}")
                bb = work.tile([P, ht, h], BF, tag=f"bb{n}")
                for mt in range(ht):
                    nc.vector.tensor_copy(bai[:, mt, :], Ai.row(mt, 0, h))
                    nc.vector.tensor_copy(bb[:, mt, :], B.row(mt, 0, h))
                mm512(MV(bai), MV(bb),
                      lambda ps, mt, c0, w: nc.scalar.copy(G.row(mt, c0, w),
                                                           ps),
                      ht, h, ht)
                Rv = MV(sh_t)  # S's storage is free until S is formed
                mm512(A, G,
                      lambda ps, mt, c0, w: nc.vector.scalar_tensor_tensor(
                          Rv.row(mt, c0, w), ps, -1.0, B.row(mt, c0, w),
                          op0=MULT, op1=ADD),
                      ht, h, ht)
                mm512(Ai, Rv,
                      lambda ps, mt, c0, w: nc.vector.tensor_add(
                          G.row(mt, c0, w), ps, G.row(mt, c0, w)),
                      ht, h, ht)
                # S = D - B^T G
                mm512(B, G,
                      lambda ps, mt, c0, w: nc.vector.scalar_tensor_tensor(
                          S.row(mt, c0, w), ps, -1.0, Dd.row(mt, c0, w),
                          op0=MULT, op1=ADD),
                      ht, h, ht)
                emit_inv(h, S, Si)
                # G <- G^T  (S dead -> its tile becomes H's storage)
                emit_transpose_inplace(G, ht)
                Gt = G
                H = MV(sh_t)
                # H = Si @ G^T
                mm512(Si, Gt,
                      lambda ps, mt, c0, w: nc.scalar.copy(H.row(mt, c0, w),
                                                           ps),
                      ht, h, ht)
                # TL = Ai + G @ H   (in place: Ai lives in ov(0,0))
                mm512(Gt, H,
                      lambda ps, mt, c0, w: nc.vector.tensor_add(
                          Ai.row(mt, c0, w), ps, Ai.row(mt, c0, w)),
                      ht, h, ht)
                # TR = -(G @ Si) ; BL = -H
                mm512(Gt, Si,
                      lambda ps, mt, c0, w: nc.vector.tensor_scalar_mul(
                          ov.sub(0, h).row(mt, c0, w), ps, -1.0),
                      ht, h, ht)
                for mt in range(ht):
                    nc.scalar.mul(ov.sub(h, 0).row(mt, 0, h),
                                  H.row(mt, 0, h), -1.0)

            if dump_minv:
                for t in range(PT):
                    nc.sync.dma_start(o_dbg[t * P:(t + 1) * P, :],
                                      mmat[:, t, :])

            emit_inv(N, MV(mmat), MV(minv))

            if dump_minv:
                for t in range(PT):
                    nc.sync.dma_start(o_dbg[N + t * P:N + (t + 1) * P, :],
                                      minv[:, t, :])

            # ---- 8-rank AllGather; core c contributes the top (c<4) or
            # bottom (c>=4) half of its Minv, selected by a data-driven
            # blend so the program stays uniform.  ag_out slot k holds
            # Minv_k rows [0:512), slot k+4 holds rows [512:1024).
            for a in range(2):
                stage = mstr.tile([P, 2, N], FP, tag="mstrip")
                nc.vector.tensor_scalar_mul(stage[:],
                                            minv[:, 4 + 2 * a:6 + 2 * a, :],
                                            cs(C_HSEL1))
                nc.vector.scalar_tensor_tensor(
                    stage[:], minv[:, 2 * a:2 * a + 2, :], cs(C_HSEL),
                    stage[:], op0=MULT, op1=ADD)
                nc.sync.dma_start(
                    ag_in.ap()[a * 256:(a + 1) * 256, :]
                    .rearrange("(t p) c -> p t c", p=P),
                    stage[:])
            nc.gpsimd.collective_compute(
                "AllGather", mybir.AluOpType.bypass,
                replica_groups=[list(range(NC))],
                ins=[ag_in.ap().opt()],
                outs=[ag_out.ap().opt()],
            )

            # ---- CxZ = [Cx[:, 0:128] | Z1] ----
            cxz = work.tile([P, PT, UW], FP)
            for t in range(PT):
                nc.vector.tensor_copy(cxz[:, t, 0:P], cx[:, t, 0:P])
                nc.vector.tensor_copy(cxz[:, t, P:UW], z1[:, t, :])

            # ---- U_k = Minv_k @ CxZ ;  T_k = Cx^T @ U_k[:, :128] ----
            # Minv_k streamed from the gather buffer in two half strips
            # (contiguous 2 MiB DMAs); combined tile [*, k*PT+mt, 0:UW]=U,
            # [*, k*PT+mt, UW:UTW]=T.
            ut = big.tile([P, L * PT, UTW], FP, tag="minv")
            for k in range(L):
                for q in range(2):
                    strip = mstr.tile([P, 4, N], FP, tag="mstrip")
                    r0 = (k + 4 * q) * 512
                    src = ag_out.ap()[r0:r0 + 512, :]
                    nc.sync.dma_start(
                        strip[:],
                        src.rearrange("(a p) c -> p a c", p=P))
                    for mt in range(PT):
                        ps = ps132.tile([P, UW], FP, tag="umm")
                        for kl in range(4):
                            nc.tensor.matmul(
                                ps[:], strip[:, kl, mt * P:(mt + 1) * P],
                                cxz[:, q * 4 + kl, :],
                                start=(kl == 0), stop=(kl == 3))
                        if q == 0:
                            nc.scalar.copy(ut[:, k * PT + mt, 0:UW], ps[:])
                        else:
                            nc.vector.tensor_add(ut[:, k * PT + mt, 0:UW],
                                                 ps[:],
                                                 ut[:, k * PT + mt, 0:UW])
                for mt in range(PT):
                    ps = ps128.tile([P, P], FP, tag="mm128")
                    for kt in range(PT):
                        nc.tensor.matmul(ps[:],
                                         cx[:, kt, mt * P:(mt + 1) * P],
                                         ut[:, k * PT + kt, 0:P],
                                         start=(kt == 0), stop=(kt == PT - 1))
                    nc.scalar.copy(ut[:, k * PT + mt, UW:UTW], ps[:])

            # ---- V = Z2 @ (W^T task_K) ;  mean = Cx[:, :128]^T @ V ----
            v = work.tile([P, PT, L], FP)
            for i in range(L):
                for k in range(L):
                    z2k = ut[:, k * PT:(k + 1) * PT, P + k]
                    if k == 0:
                        nc.vector.tensor_scalar_mul(v[:, :, i], z2k,
                                                    cs(C_WT2 + 4 * k + i))
                    else:
                        nc.vector.scalar_tensor_tensor(
                            v[:, :, i], z2k, cs(C_WT2 + 4 * k + i), v[:, :, i],
                            op0=MULT, op1=ADD)
            psm = ps128.tile([P, L], FP, tag="mm128")
            for kt in range(PT):
                nc.tensor.matmul(psm[:], cx[:, kt, 0:P], v[:, kt, :],
                                 start=(kt == 0), stop=(kt == PT - 1))
            meansb = work.tile([P, L], FP)
            nc.vector.tensor_copy(meansb[:], psm[:])
            nc.sync.dma_start(o_mean, meansb[:])

            # ---- fvar stripes: only task blocks i <= j (host mirrors).
            # Two row-tiles per op (wide APs) to halve instruction count;
            # acc tiles reuse the dead top-level Schur scratch slots.
            for i in range(L):
                for tp in range(PT // 2):
                    wj = (L - i) * P
                    tag = "g1024" if tp % 2 == 0 else "sh1024"
                    acc = work.tile([P, 2, L * P], FP, tag=tag)
                    for j in range(i, L):
                        sub = acc[:, :, (j - i) * P:(j - i + 1) * P]
                        nc.scalar.mul(sub, cxx[:, 2 * tp:2 * tp + 2, :],
                                      cs(C_TK + 4 * i + j))
                        for k in range(L):
                            nc.vector.scalar_tensor_tensor(
                                sub,
                                ut[:, k * PT + 2 * tp:k * PT + 2 * tp + 2,
                                   UW:UTW],
                                cs(C_SC + 16 * i + 4 * j + k), sub,
                                op0=MULT, op1=ADD)
                    nc.sync.dma_start(
                        o_fvar[i * NT + 2 * tp * P:i * NT + (2 * tp + 2) * P,
                               i * P:i * P + wj]
                        .rearrange("(t p) c -> p t c", p=P),
                        acc[:, :, 0:wj])

    nc.compile()
    return nc


_NC_CACHE = [None]


def _get_program():
    if _NC_CACHE[0] is None:
        _NC_CACHE[0] = _build_program()
    return _NC_CACHE[0]


def _host_prep(X, test_X, Y, log_noise, covar_factor, log_var, log_lengthscale):
    X = np.asarray(X, f32)
    test_X = np.asarray(test_X, f32)
    Y = np.asarray(Y, f32)
    log_noise = np.asarray(log_noise, f32)
    covar_factor = np.asarray(covar_factor, f32)
    log_var = np.asarray(log_var, f32)
    ls = float(np.asarray(log_lengthscale, f32))

    task_K = (covar_factor @ covar_factor.T
              + np.diag(np.exp(log_var))).astype(f32)
    d = np.exp(log_noise).astype(f32)
    dih = (1.0 / np.sqrt(d)).astype(f32)
    lam, Pm = np.linalg.eigh(
        (dih[:, None] * task_K * dih[None, :]).astype(np.float64))
    lam = lam.astype(f32)
    Pm = Pm.astype(f32)
    W = (dih[:, None] * Pm).astype(f32)
    Smat = (task_K @ W).astype(f32)
    Wt2 = (W.T @ task_K).astype(f32)
    Z1 = (Y @ W).astype(f32)
    expsc = f32(-0.5 * np.exp(-2.0 * ls))

    x2 = np.sum(X * X, axis=1).astype(f32)
    tx2 = np.sum(test_X * test_X, axis=1).astype(f32)

    in_maps = []
    for c in range(NC):
        k = c % 4
        perm = np.roll(np.arange(NT), -P * c)
        trot = test_X[perm]
        tx2rot = tx2[perm]
        cstv = np.zeros((CN,), f32)
        cstv[C_LAM] = lam[k]
        cstv[C_EXPSC] = expsc
        cstv[C_HSEL] = 1.0 if c < 4 else 0.0
        cstv[C_HSEL1] = 0.0 if c < 4 else 1.0
        for i in range(L):
            for j in range(L):
                cstv[C_TK + 4 * i + j] = task_K[i, j]
                for kk in range(L):
                    cstv[C_SC + 16 * i + 4 * j + kk] = \
                        -Smat[i, kk] * Smat[j, kk]
        for kk in range(L):
            for i in range(L):
                cstv[C_WT2 + 4 * kk + i] = Wt2[kk, i]
        in_maps.append({
            "xt": np.ascontiguousarray(X.T),
            "txt": np.ascontiguousarray(trot.T),
            "x2c": np.ascontiguousarray(expsc * x2.reshape(PT, P).T),
            "x2r": x2.reshape(1, N).copy(),
            "tx2c": np.ascontiguousarray(expsc * tx2rot.reshape(PT, P).T),
            "tx2r": tx2rot.reshape(1, NT).copy(),
            "z1": Z1,
            "cst": np.broadcast_to(cstv, (P, CN)).copy(),
        })
    return in_maps, d


def _stitch(results, d):
    fvar = np.empty((L * NT, L * NT), f32)
    mean = np.empty((NT, L), f32)
    for c in range(NC):
        fp = results[c]["fvar_part"]
        mean[c * P:(c + 1) * P, :] = results[c]["mean_part"]
        for i in range(L):
            blk = np.roll(fp[i * NT:(i + 1) * NT, :], P * c, axis=0)
            for j in range(i, L):
                fvar[i * NT:(i + 1) * NT,
                     j * NT + c * P:j * NT + (c + 1) * P] = \
                    blk[:, j * P:(j + 1) * P]
    for i in range(1, L):
        for j in range(i):
            fvar[i * NT:(i + 1) * NT, j * NT:(j + 1) * NT] = \
                fvar[j * NT:(j + 1) * NT, i * NT:(i + 1) * NT].T
    noise = np.zeros((L * N, L * N), f32)
    noise[np.arange(L * N), np.arange(L * N)] = np.repeat(d, N)
    fmean = mean.T.reshape(-1, 1).copy()
    return fmean, fvar, noise, mean


def kernel(**inputs):
    nc = _get_program()
    in_maps, d = _host_prep(**inputs)
    res = bass_utils.run_bass_kernel_spmd(nc, in_maps,
                                          core_ids=list(range(NC)))
    return _stitch(res.results, d)
